# revision 18
# baseline (speedup 1.0000x reference)
"""Trainium2 Bass kernel for nn_BinaryTreeShInvariantConv.

Per (b, v): gather P=32 neighbor rows of signal[b] (Cin=64), contract over P
against conv_kernel[b,v] -> y[Cin, R*N], square, sum SH orders per degree l,
sqrt(+eps), contract [Cin*R*(L+1)=512] against kernel_weights -> [Cout=128],
bias + relu.

Sharding: data-parallel over batch B=8 -> one batch per NeuronCore (SPMD).

Dataflow per core (one batch, V=4096), in supergroups of 128 v's:
  - patches arrive either via on-device dma_gather (PREGATHER=False) or as a
    host-pregathered dense stream pp[pair, (j,p), (sq, g, c)] (PREGATHER=True;
    same HBM bytes, but dense 8KB descriptors instead of random 128B rows,
    which halves the DMA-engine time and frees the GPSIMD queue).
  - Kbd: block-diagonal conv_kernel [128 (j,p), (j' 4, sq 2, g 32, rn 32)];
    off-diagonal zeros memset once and persistent; diagonal refilled every
    KGRP supergroups by 4 contiguous [32, 4KB] DMAs.
  - MM1 per 4-v chunk g (h=g//16, bank=(g//8)%2, blk=g%8): stationary
    lhsT = patches chunk [128, 64 c], moving rhs = kbd strided slice
    [128, (j' 4, rn 32)] -> psum ps1[bank][64h:, 128blk:] = [64 c, 128 (j,rn)].
    This (h, bank, blk) split makes ps3 partitions linear in v (single store).
  - square (ACT x2 banks) -> ysq bf16 [128 (h,c), (bank, blk, j, rn) 2048].
  - degree sums over n-windows: l=2,3 DVE reduce_sum; l=1 Pool adds;
    l=0 Pool copy -> zpre f32 sbuf [128, (a, b, l) 512].
  - sqrt(x + 1e-4) on ACT -> zsb bf16 (x >= 0 so eps-add ~= max(x, eps)).
  - MM3 per (h, rl): lhsT = zsb slice [64 c, 64 (a,j)], rhs = W [64, 128 i],
    accumulate 8 rl in psum -> ps3 [(h,bank,blk,j)=v 128, 128 i].
  - relu (+ bias if nonzero) on DVE -> osb; one [128, 512B-row] store per sg.
"""

import sys

sys.path.insert(0, "/opt/trn_rl_repo")

import numpy as np

import concourse.bacc as bacc
import concourse.mybir as mybir
import concourse.tile as tile
from concourse import bass2jax

B, V, P, CIN, R, COUT = 8, 4096, 32, 64, 2, 128
NSH, NDEG = 16, 4
VSG = 128            # v's per supergroup
NSG = V // VSG       # 32 supergroups
NCHUNK = VSG // 4    # 32 chunks of 4 v's
SGI = VSG * P        # 4096 gather indices per supergroup
GSZ = 4096           # indices per dma_gather op
KGRP = 4             # supergroups per kbd/patch tile refill
PREGATHER = True     # host-side signal[pidx]: dense stream vs random gather
BF16 = mybir.dt.bfloat16
F32 = mybir.dt.float32
I16 = mybir.dt.int16

_CACHE = {}


def _dma_gather_any(eng, out_ap, in_ap, idxs_ap, num_idxs, elem_size,
                    single_packet=True):
    """bass.dma_gather minus the elem_size%256 assert (the Q7 ucode only
    requires the source ROW STRIDE to be a 256B multiple; the bytes read per
    row are free). in_ap's outer stride (elem_step) must be 256B-aligned."""
    from concourse import ap_utils
    from concourse.bass import MemorySpace

    assert idxs_ap.dtype == I16
    assert in_ap.space == MemorySpace.DRAM
    assert in_ap.dtype == out_ap.dtype
    elem_step = in_ap.ap[0][0]
    stride_bytes = elem_step * mybir.dt.size(in_ap.dtype)
    assert stride_bytes % 256 == 0 and stride_bytes // 256 < 256
    assert ap_utils.ap_is_contiguous(out_ap.ap[1:])
    assert ap_utils.ap_is_contiguous(idxs_ap.ap[1:])
    assert in_ap.ap[-1][1] == out_ap.ap[-1][1] == elem_size
    assert out_ap.ap[0][1] * out_ap.ap[1][1] == ((num_idxs + 127) // 128) * 128

    _in_ap = eng.lower_ap_dma(in_ap, for_custom_bir_dma=True)
    return eng.add_instruction(
        mybir.InstDMAGatherAnt(
            name=eng.bass.get_next_instruction_name(),
            ins=[*_in_ap, eng.lower_ap(idxs_ap),
                 eng.lower_val_access(eng.to_reg(num_idxs))],
            outs=[eng.lower_ap(out_ap)],
            transpose=False,
            num_idxs=num_idxs,
            elem_size=elem_size,
            stride_bytes_256=stride_bytes // 256,
            gen_mode=0,
            single_packet=single_packet,
            queue_num=0,
            sbuf_tokens_per_rank=0,
            sbuf_free_dim_per_rank=0,
            sbuf_free_dim_pad_per_rank=0,
            sbuf_byte_offset=0,
        ))


def _build_nc(nsg, with_bias):
    nc = bacc.Bacc("TRN2", target_bir_lowering=False, debug=False,
                   enable_asserts=False)
    vtot = nsg * VSG
    ngrp = nsg // KGRP
    kcols = KGRP * NCHUNK * R * NSH          # cols per j block: (sq, g, rn)
    if PREGATHER:
        pp = nc.dram_tensor("pp", [ngrp, 128, KGRP * NCHUNK * CIN], BF16,
                            kind="ExternalInput")
    else:
        sig = nc.dram_tensor("sig", [V, 128], BF16, kind="ExternalInput")
        idx = nc.dram_tensor("idx", [128, (SGI // 16) * nsg], I16,
                             kind="ExternalInput")
    kre = nc.dram_tensor("kre", [ngrp, 128, kcols], BF16,
                         kind="ExternalInput")
    wsb = nc.dram_tensor("wsb", [128, 8 * COUT], BF16, kind="ExternalInput")
    bia = nc.dram_tensor("bia", [1, COUT], F32, kind="ExternalInput")
    outd = nc.dram_tensor("outd", [vtot, COUT], F32, kind="ExternalOutput")

    AF = mybir.ActivationFunctionType
    ALU = mybir.AluOpType
    with tile.TileContext(nc) as tc:
        with (
            tc.tile_pool(name="const", bufs=1) as constp,
            tc.tile_pool(name="kbd", bufs=3) as stgp,
            tc.tile_pool(name="patches", bufs=3) as patp,
            tc.tile_pool(name="ysq", bufs=2) as ysqp,
            tc.tile_pool(name="zpre", bufs=2) as zprep,
            tc.tile_pool(name="zsb", bufs=3) as zsbp,
            tc.tile_pool(name="osb", bufs=3) as osbp,
            tc.tile_pool(name="ps1", bufs=3, space="PSUM") as ps1p,
            tc.tile_pool(name="ps3", bufs=2, space="PSUM") as ps3p,
        ):
            w_t = constp.tile([128, 8 * COUT], BF16, tag="w")
            nc.sync.dma_start(w_t[:], wsb.ap())
            if not PREGATHER:
                idx_t = constp.tile([128, (SGI // 16) * nsg], I16, tag="idx")
                nc.sync.dma_start(idx_t[:], idx.ap())
            if with_bias:
                bias_t = constp.tile([1, COUT], F32, tag="bias")
                nc.sync.dma_start(bias_t[:], bia.ap())

            eps_t = constp.tile([128, 1], F32, tag="eps")
            nc.vector.memset(eps_t[:], 1e-4)

            stg_tiles = {}

            zsb2s = [zsbp.tile([128, 8 * COUT], BF16, tag="zsb2",
                               name=f"zsb2_{i}") for i in range(3)]
            for i in range(3):
                nc.vector.memset(zsb2s[i][:], 0.0)

            pat_tiles = {}

            def issue_pat(g, split=False):
                t = patp.tile([128, KGRP, NCHUNK, CIN], BF16, tag="pat",
                              name=f"pat{g % 3}")
                pat_tiles[g] = t
                if split:
                    nc.sync.dma_start(t[:, 0], pp.ap()[g].rearrange(
                        "p (sq r) -> p sq r", sq=KGRP)[:, 0])
                    nc.sync.dma_start(
                        t[:, 1:], pp.ap()[g].rearrange(
                            "p (sq r) -> p sq r", sq=KGRP)[:, 1:])
                else:
                    nc.sync.dma_start(t[:], pp.ap()[g])

            def issue_kbd(g):
                t = stgp.tile([128, kcols], BF16, tag="stg",
                              name=f"stg{g % 3}")
                stg_tiles[g] = t
                nc.sync.dma_start(t[:], kre.ap()[g])

            if PREGATHER:
                issue_pat(0, split=True)
                issue_pat(1)
            issue_kbd(0)
            issue_kbd(1)

            prev_store = None
            for sg in range(nsg):
                grp, sq = sg // KGRP, sg % KGRP
                if sq == 0 and grp + 2 < ngrp:
                    issue_kbd(grp + 2)
                stg_r = stg_tiles[grp][:, :].rearrange(
                    "p (sq g rn) -> p sq g rn", sq=KGRP, g=NCHUNK)

                # --- patches ----------------------------------------------
                if PREGATHER:
                    if sq == 0 and grp + 2 < ngrp:
                        issue_pat(grp + 2)
                    pat = pat_tiles[grp][:, sq]
                else:
                    patg = patp.tile([128, NCHUNK, CIN], BF16, tag="pat")
                    pat_r = patg[:, :, :].rearrange("p (t u) c -> p t u c",
                                                    t=1)
                    _dma_gather_any(
                        nc.gpsimd, pat_r[:, 0], sig.ap()[:, 0:CIN],
                        idx_t[:, (SGI // 16) * sg:(SGI // 16) * (sg + 1)],
                        GSZ, CIN, single_packet=False)
                    pat = patg[:, :, :]

                # store for the previous sg (data long ready; avoids
                # blocking any queue on this sg's compute chain)
                if prev_store is not None:
                    nc.gpsimd.dma_start(*prev_store)
                    prev_store = None

                # --- MM1: chunk g = 16h + 8b + blk -> ps1[b][64h, 128blk] -
                ps1 = [ps1p.tile([128, 1024], F32, tag="ps1",
                                 name=f"ps1_{sg}_{b}") for b in range(2)]
                for b in range(2):
                    for h in range(2):
                        for blk in range(8):
                            g = 16 * h + 8 * b + blk
                            for j in range(4):
                                nc.tensor.matmul(
                                    ps1[b][64 * h:64 * (h + 1),
                                           128 * blk + 32 * j:
                                           128 * blk + 32 * (j + 1)],
                                    pat[32 * j:32 * (j + 1), g, :],
                                    stg_r[32 * j:32 * (j + 1), sq, g, :],
                                    start=True, stop=True,
                                    tile_position=(32 * j, 64 * h))

                # --- square: both banks on ACT (HW allows only one PSUM
                # input per DVE/Pool tensor op, so DVE cannot square psum) --
                ysq = ysqp.tile([128, 2048], BF16, tag="ysq")
                nc.scalar.activation(ysq[:, 0:1024], ps1[0][:], AF.Square)
                nc.scalar.activation(ysq[:, 1024:2048], ps1[1][:], AF.Square)

                # --- degree sums over n-windows ---------------------------
                # ysq free: (a=(b,blk) 16, bb=(j,r) 8, n 16)
                zpre = zprep.tile([128, 512], F32, tag="zpre")
                ysq_r = ysq[:, :].rearrange("p (a bb n) -> p a bb n",
                                            a=16, bb=8)
                zpre_r = zpre[:, :].rearrange("p (a bb l) -> p a bb l",
                                              a=16, bb=8)
                # l=0: plain copy (Pool)
                nc.gpsimd.tensor_copy(zpre_r[:, :, :, 0], ysq_r[:, :, :, 0])
                # l=1: two adds (Pool)
                nc.gpsimd.tensor_tensor(zpre_r[:, :, :, 1],
                                        ysq_r[:, :, :, 1],
                                        ysq_r[:, :, :, 2], ALU.add)
                nc.gpsimd.tensor_tensor(zpre_r[:, :, :, 1],
                                        zpre_r[:, :, :, 1],
                                        ysq_r[:, :, :, 3], ALU.add)
                # l=2: DVE window reduce
                nc.vector.reduce_sum(
                    zpre_r[:, :, :, 2], ysq_r[:, :, :, 4:9],
                    axis=mybir.AxisListType.X)
                # l=3: DVE window reduce
                nc.vector.reduce_sum(
                    zpre_r[:, :, :, 3], ysq_r[:, :, :, 9:16],
                    axis=mybir.AxisListType.X)

                # --- sqrt into block-diag zsb2[(h,c), (r,l,v)]; the
                # off-block zeros persist in the pinned tiles, so each rl
                # slice is a [128, 128] single-free-dim lhsT with K=(h,c) ---
                zsb2 = zsb2s[sg % 3]
                zpre_h = zpre[:, :].rearrange("p (a j r l) -> p a j r l",
                                              a=16, j=4, r=2)
                zsb2_h = zsb2[:, :].rearrange(
                    "p (r l v2 a j) -> p a j r l v2", r=2, l=4, v2=2, a=16)
                for h in range(2):
                    pa, pb = 64 * h, 64 * (h + 1)
                    nc.scalar.activation(
                        zsb2_h[pa:pb, :, :, :, :, h],
                        zpre_h[pa:pb, :, :, :, :], AF.Sqrt)

                # --- MM3: contract (h, c) x 8 rl against duplicated W -----
                ps3 = ps3p.tile([128, COUT], F32, tag="ps3")
                for rl in range(8):
                    nc.tensor.matmul(
                        ps3[:, :],
                        zsb2[:, COUT * rl:COUT * (rl + 1)],
                        w_t[:, COUT * rl:COUT * (rl + 1)],
                        start=(rl == 0), stop=(rl == 7),
                        skip_group_check=True)

                # --- bias + relu; store deferred one sg -------------------
                osb = osbp.tile([128, COUT], F32, tag="osb")
                if with_bias:
                    nc.vector.tensor_add(
                        osb[:], ps3[:],
                        bias_t[:, :].broadcast(0, 128))
                    nc.vector.tensor_scalar_max(osb[:], osb[:], 0.0)
                else:
                    nc.vector.tensor_scalar_max(osb[:], ps3[:], 0.0)
                prev_store = (outd.ap()[VSG * sg:VSG * (sg + 1), :], osb[:])
            nc.gpsimd.dma_start(*prev_store)

    nc.compile()
    return nc


def _prep_inputs_core(b, signal, patches_idx, conv_kernel, kernel_weights,
                      biases, nsg):
    bf = mybir.dt.np(BF16)
    ngrp = nsg // KGRP
    # kre[grp, j, p, (sq, g, rn)] = conv_kernel[b, (grp*KGRP+sq)*128+4g+j, p, rn]
    k = conv_kernel[b].reshape(ngrp, KGRP, NCHUNK, 4, P, R * NSH)
    kre = np.ascontiguousarray(
        k.transpose(0, 3, 4, 1, 2, 5)).reshape(
        ngrp, 128, KGRP * NCHUNK * R * NSH).astype(bf)
    # wsb[c + 64*dup, rl*128 + i] = kernel_weights[i, c, r, l], rl = 4r + l
    w = kernel_weights.transpose(2, 3, 1, 0).reshape(8, CIN, COUT)
    wrow = np.ascontiguousarray(w.transpose(1, 0, 2)).reshape(CIN, 8 * COUT)
    wsb = np.concatenate([wrow, wrow], axis=0).astype(bf)
    bia = biases.reshape(1, COUT).astype(np.float32)
    out = {"kre": kre, "wsb": wsb, "bia": bia}
    if PREGATHER:
        # pp[grp, 32j+p, (sq, g, c)] = signal[b][pidx[v=(grp*KGRP+sq)*128+4g+j, p]]
        pb = signal[b].astype(bf)[patches_idx[b, :, :, 1]]   # [V, P, C] bf16
        pb = pb.reshape(ngrp, KGRP, NCHUNK, 4, P, CIN)
        out["pp"] = np.ascontiguousarray(
            pb.transpose(0, 3, 4, 1, 2, 5)).reshape(
            ngrp, 128, KGRP * NCHUNK * CIN)
    else:
        sig = np.zeros((V, 128), dtype=bf)
        sig[:, :CIN] = signal[b].astype(bf)
        out["sig"] = sig
        out["idx"] = _fix_idx_wrap(
            patches_idx[b, :, :, 1].astype(np.int16).reshape(-1))
    return out


def _fix_idx_wrap(pidx_flat):
    # wrap order is per gather op: each op's GSZ idxs wrapped into 16
    # partitions independently.
    blk = pidx_flat.reshape(-1, GSZ // 16, 16)   # [ops, GSZ/16, 16]
    out = np.ascontiguousarray(
        blk.transpose(0, 2, 1).transpose(1, 0, 2)).reshape(16, -1)
    return np.tile(out, (8, 1))


def _make_runner(nc, n_cores=8):
    import jax
    from jax.sharding import Mesh, PartitionSpec
    from jax.experimental.shard_map import shard_map

    bass2jax.install_neuronx_cc_hook()
    partition_name = (nc.partition_id_tensor.name
                      if nc.partition_id_tensor else None)
    in_names, out_names, out_avals, zero_outs = [], [], [], []
    for alloc in nc.m.functions[0].allocations:
        if not isinstance(alloc, mybir.MemoryLocationSet):
            continue
        name = alloc.memorylocations[0].name
        if alloc.kind == "ExternalInput":
            if name != partition_name:
                in_names.append(name)
        elif alloc.kind == "ExternalOutput":
            out_names.append(name)
            shape = tuple(alloc.tensor_shape)
            dtype = mybir.dt.np(alloc.dtype)
            out_avals.append(jax.core.ShapedArray(shape, dtype))
            zero_outs.append(np.zeros(shape, dtype))
    n_params, n_outs = len(in_names), len(out_avals)
    in_names_all = list(in_names) + list(out_names)
    if partition_name is not None:
        in_names_all.append(partition_name)

    def _body(*args):
        operands = list(args)
        if partition_name is not None:
            operands.append(bass2jax.partition_id_tensor())
        outs = bass2jax._bass_exec_p.bind(
            *operands, out_avals=tuple(out_avals),
            in_names=tuple(in_names_all), out_names=tuple(out_names),
            lowering_input_output_aliases=(),
            sim_require_finite=True, sim_require_nnan=True, nc=nc)
        return tuple(outs)

    donate = tuple(range(n_params, n_params + n_outs))
    devices = jax.devices()[:n_cores]
    mesh = Mesh(np.asarray(devices), ("core",))
    sharded = jax.jit(
        shard_map(_body, mesh=mesh,
                  in_specs=(PartitionSpec("core"),) * (n_params + n_outs),
                  out_specs=(PartitionSpec("core"),) * n_outs,
                  check_rep=False),
        donate_argnums=donate, keep_unused=True)

    def run_fn(in_maps):
        import jax
        per_core = [[np.asarray(m[nm]) for nm in in_names] for m in in_maps]
        concat_in = [
            np.concatenate([per_core[c][i] for c in range(n_cores)], axis=0)
            for i in range(n_params)]
        concat_zeros = [
            np.zeros((n_cores * z.shape[0], *z.shape[1:]), z.dtype)
            for z in zero_outs]
        out_arrs = sharded(*concat_in, *concat_zeros)
        jax.block_until_ready(out_arrs)
        return [
            {nm: np.asarray(out_arrs[i]).reshape(n_cores, *out_avals[i].shape)[c]
             for i, nm in enumerate(out_names)}
            for c in range(n_cores)]

    return run_fn


def kernel(signal, patches_idx, conv_kernel, kernel_weights, biases):
    with_bias = bool(np.any(biases))
    key = ("k", NSG, with_bias)
    if key not in _CACHE:
        nc = _build_nc(NSG, with_bias)
        _CACHE[key] = (nc, _make_runner(nc))
    nc, run = _CACHE[key]

    in_maps = []
    for b in range(B):
        m = _prep_inputs_core(b, signal, patches_idx, conv_kernel,
                              kernel_weights, biases, NSG)
        in_maps.append(m)

    results = run(in_maps)
    out = np.stack([results[b]["outd"] for b in range(B)], axis=0)
    return out.astype(np.float32)


# revision 19
# speedup vs baseline: 1.1647x; 1.1647x over previous
"""Trainium2 Bass kernel for nn_BinaryTreeShInvariantConv.

Per (b, v): gather P=32 neighbor rows of signal[b] (Cin=64), contract over P
against conv_kernel[b,v] -> y[Cin, R*N], square, sum SH orders per degree l,
sqrt(+eps), contract [Cin*R*(L+1)=512] against kernel_weights -> [Cout=128],
bias + relu.

Sharding: data-parallel over batch B=8 -> one batch per NeuronCore (SPMD).

Dataflow per core (one batch, V=4096), in supergroups of 128 v's:
  - patches arrive either via on-device dma_gather (PREGATHER=False) or as a
    host-pregathered dense stream pp[pair, (j,p), (sq, g, c)] (PREGATHER=True;
    same HBM bytes, but dense 8KB descriptors instead of random 128B rows,
    which halves the DMA-engine time and frees the GPSIMD queue).
  - Kbd: block-diagonal conv_kernel [128 (j,p), (j' 4, sq 2, g 32, rn 32)];
    off-diagonal zeros memset once and persistent; diagonal refilled every
    KGRP supergroups by 4 contiguous [32, 4KB] DMAs.
  - MM1 per 4-v chunk g (h=g//16, bank=(g//8)%2, blk=g%8): stationary
    lhsT = patches chunk [128, 64 c], moving rhs = kbd strided slice
    [128, (j' 4, rn 32)] -> psum ps1[bank][64h:, 128blk:] = [64 c, 128 (j,rn)].
    This (h, bank, blk) split makes ps3 partitions linear in v (single store).
  - square (ACT x2 banks) -> ysq bf16 [128 (h,c), (bank, blk, j, rn) 2048].
  - degree sums over n-windows: l=2,3 DVE reduce_sum; l=1 Pool adds;
    l=0 Pool copy -> zpre f32 sbuf [128, (a, b, l) 512].
  - sqrt(x + 1e-4) on ACT -> zsb bf16 (x >= 0 so eps-add ~= max(x, eps)).
  - MM3 per (h, rl): lhsT = zsb slice [64 c, 64 (a,j)], rhs = W [64, 128 i],
    accumulate 8 rl in psum -> ps3 [(h,bank,blk,j)=v 128, 128 i].
  - relu (+ bias if nonzero) on DVE -> osb; one [128, 512B-row] store per sg.
"""

import sys

sys.path.insert(0, "/opt/trn_rl_repo")

import numpy as np

import concourse.bacc as bacc
import concourse.mybir as mybir
import concourse.tile as tile
from concourse import bass2jax

B, V, P, CIN, R, COUT = 8, 4096, 32, 64, 2, 128
NSH, NDEG = 16, 4
VSG = 128            # v's per supergroup
NSG = V // VSG       # 32 supergroups
NCHUNK = VSG // 4    # 32 chunks of 4 v's
SGI = VSG * P        # 4096 gather indices per supergroup
GSZ = 4096           # indices per dma_gather op
KGRP = 4             # supergroups per kbd/patch tile refill
PREGATHER = True     # host-side signal[pidx]: dense stream vs random gather
BF16 = mybir.dt.bfloat16
F32 = mybir.dt.float32
I16 = mybir.dt.int16

_CACHE = {}


def _dma_gather_any(eng, out_ap, in_ap, idxs_ap, num_idxs, elem_size,
                    single_packet=True):
    """bass.dma_gather minus the elem_size%256 assert (the Q7 ucode only
    requires the source ROW STRIDE to be a 256B multiple; the bytes read per
    row are free). in_ap's outer stride (elem_step) must be 256B-aligned."""
    from concourse import ap_utils
    from concourse.bass import MemorySpace

    assert idxs_ap.dtype == I16
    assert in_ap.space == MemorySpace.DRAM
    assert in_ap.dtype == out_ap.dtype
    elem_step = in_ap.ap[0][0]
    stride_bytes = elem_step * mybir.dt.size(in_ap.dtype)
    assert stride_bytes % 256 == 0 and stride_bytes // 256 < 256
    assert ap_utils.ap_is_contiguous(out_ap.ap[1:])
    assert ap_utils.ap_is_contiguous(idxs_ap.ap[1:])
    assert in_ap.ap[-1][1] == out_ap.ap[-1][1] == elem_size
    assert out_ap.ap[0][1] * out_ap.ap[1][1] == ((num_idxs + 127) // 128) * 128

    _in_ap = eng.lower_ap_dma(in_ap, for_custom_bir_dma=True)
    return eng.add_instruction(
        mybir.InstDMAGatherAnt(
            name=eng.bass.get_next_instruction_name(),
            ins=[*_in_ap, eng.lower_ap(idxs_ap),
                 eng.lower_val_access(eng.to_reg(num_idxs))],
            outs=[eng.lower_ap(out_ap)],
            transpose=False,
            num_idxs=num_idxs,
            elem_size=elem_size,
            stride_bytes_256=stride_bytes // 256,
            gen_mode=0,
            single_packet=single_packet,
            queue_num=0,
            sbuf_tokens_per_rank=0,
            sbuf_free_dim_per_rank=0,
            sbuf_free_dim_pad_per_rank=0,
            sbuf_byte_offset=0,
        ))


def _build_nc(nsg, with_bias):
    nc = bacc.Bacc("TRN2", target_bir_lowering=False, debug=False,
                   enable_asserts=False)
    vtot = nsg * VSG
    ngrp = nsg // KGRP
    kcols = KGRP * NCHUNK * R * NSH          # cols per j block: (sq, g, rn)
    if PREGATHER:
        pp = nc.dram_tensor("pp", [ngrp, 128, KGRP * NCHUNK * CIN], BF16,
                            kind="ExternalInput")
    else:
        sig = nc.dram_tensor("sig", [V, 128], BF16, kind="ExternalInput")
        idx = nc.dram_tensor("idx", [128, (SGI // 16) * nsg], I16,
                             kind="ExternalInput")
    kre = nc.dram_tensor("kre", [ngrp, 128, kcols], BF16,
                         kind="ExternalInput")
    wsb = nc.dram_tensor("wsb", [128, 8 * COUT], BF16, kind="ExternalInput")
    bia = nc.dram_tensor("bia", [1, COUT], F32, kind="ExternalInput")
    outd = nc.dram_tensor("outd", [vtot, COUT], F32, kind="ExternalOutput")

    AF = mybir.ActivationFunctionType
    ALU = mybir.AluOpType
    with tile.TileContext(nc) as tc:
        with (
            tc.tile_pool(name="const", bufs=1) as constp,
            tc.tile_pool(name="kbd", bufs=3) as stgp,
            tc.tile_pool(name="patches", bufs=3) as patp,
            tc.tile_pool(name="ysq", bufs=2) as ysqp,
            tc.tile_pool(name="zpre", bufs=2) as zprep,
            tc.tile_pool(name="zsb", bufs=3) as zsbp,
            tc.tile_pool(name="osb", bufs=3) as osbp,
            tc.tile_pool(name="ps1", bufs=3, space="PSUM") as ps1p,
            tc.tile_pool(name="ps3", bufs=2, space="PSUM") as ps3p,
        ):
            w_t = constp.tile([128, 8 * COUT], BF16, tag="w")
            nc.sync.dma_start(w_t[:], wsb.ap())
            if not PREGATHER:
                idx_t = constp.tile([128, (SGI // 16) * nsg], I16, tag="idx")
                nc.sync.dma_start(idx_t[:], idx.ap())
            if with_bias:
                bias_t = constp.tile([1, COUT], F32, tag="bias")
                nc.sync.dma_start(bias_t[:], bia.ap())

            eps_t = constp.tile([128, 1], F32, tag="eps")
            nc.vector.memset(eps_t[:], 1e-4)

            stg_tiles = {}

            zsb2s = [zsbp.tile([128, 8 * COUT], BF16, tag="zsb2",
                               name=f"zsb2_{i}") for i in range(3)]
            for i in range(3):
                nc.vector.memset(zsb2s[i][:], 0.0)

            pat_tiles = {}

            def issue_pat(g, split=False):
                t = patp.tile([128, KGRP, NCHUNK, CIN], BF16, tag="pat",
                              name=f"pat{g % 3}")
                pat_tiles[g] = t
                if split:
                    nc.sync.dma_start(t[:, 0], pp.ap()[g].rearrange(
                        "p (sq r) -> p sq r", sq=KGRP)[:, 0])
                    nc.sync.dma_start(
                        t[:, 1:], pp.ap()[g].rearrange(
                            "p (sq r) -> p sq r", sq=KGRP)[:, 1:])
                else:
                    nc.sync.dma_start(t[:], pp.ap()[g])

            def issue_kbd(g):
                t = stgp.tile([128, kcols], BF16, tag="stg",
                              name=f"stg{g % 3}")
                stg_tiles[g] = t
                nc.sync.dma_start(t[:], kre.ap()[g])

            if PREGATHER:
                issue_pat(0, split=True)
                issue_pat(1)
            issue_kbd(0)
            issue_kbd(1)

            prev_store = None
            for sg in range(nsg):
                grp, sq = sg // KGRP, sg % KGRP
                if sq == 0 and grp + 2 < ngrp:
                    issue_kbd(grp + 2)
                stg_r = stg_tiles[grp][:, :].rearrange(
                    "p (sq g rn) -> p sq g rn", sq=KGRP, g=NCHUNK)

                # --- patches ----------------------------------------------
                if PREGATHER:
                    if sq == 0 and grp + 2 < ngrp:
                        issue_pat(grp + 2)
                    pat = pat_tiles[grp][:, sq]
                else:
                    patg = patp.tile([128, NCHUNK, CIN], BF16, tag="pat")
                    pat_r = patg[:, :, :].rearrange("p (t u) c -> p t u c",
                                                    t=1)
                    _dma_gather_any(
                        nc.gpsimd, pat_r[:, 0], sig.ap()[:, 0:CIN],
                        idx_t[:, (SGI // 16) * sg:(SGI // 16) * (sg + 1)],
                        GSZ, CIN, single_packet=False)
                    pat = patg[:, :, :]

                # store for the previous sg (data long ready; avoids
                # blocking any queue on this sg's compute chain)
                if prev_store is not None:
                    nc.gpsimd.dma_start(*prev_store)
                    prev_store = None

                # --- MM1: chunk g = 16h + 8b + blk -> ps1[b][64h, 128blk] -
                ps1 = [ps1p.tile([128, 1024], F32, tag="ps1",
                                 name=f"ps1_{sg}_{b}") for b in range(2)]
                for b in range(2):
                    for h in range(2):
                        for blk in range(8):
                            g = 16 * h + 8 * b + blk
                            for j in range(4):
                                nc.tensor.matmul(
                                    ps1[b][64 * h:64 * (h + 1),
                                           128 * blk + 32 * j:
                                           128 * blk + 32 * (j + 1)],
                                    pat[32 * j:32 * (j + 1), g, :],
                                    stg_r[32 * j:32 * (j + 1), sq, g, :],
                                    start=True, stop=True,
                                    tile_position=(32 * j, 64 * h))

                # --- square: both banks on ACT (HW allows only one PSUM
                # input per DVE/Pool tensor op, so DVE cannot square psum) --
                ysq = ysqp.tile([128, 2048], BF16, tag="ysq")
                nc.scalar.activation(ysq[:, 0:1024], ps1[0][:], AF.Square)
                nc.scalar.activation(ysq[:, 1024:2048], ps1[1][:], AF.Square)

                # --- degree sums over n-windows ---------------------------
                # ysq free: (a=(b,blk) 16, bb=(j,r) 8, n 16)
                zpre = zprep.tile([128, 512], F32, tag="zpre")
                ysq_r = ysq[:, :].rearrange("p (a bb n) -> p a bb n",
                                            a=16, bb=8)
                zpre_r = zpre[:, :].rearrange("p (a bb l) -> p a bb l",
                                              a=16, bb=8)
                # l=0: plain copy (Pool)
                nc.gpsimd.tensor_copy(zpre_r[:, :, :, 0], ysq_r[:, :, :, 0])
                # l=1: two adds (Pool)
                nc.gpsimd.tensor_tensor(zpre_r[:, :, :, 1],
                                        ysq_r[:, :, :, 1],
                                        ysq_r[:, :, :, 2], ALU.add)
                nc.gpsimd.tensor_tensor(zpre_r[:, :, :, 1],
                                        zpre_r[:, :, :, 1],
                                        ysq_r[:, :, :, 3], ALU.add)
                # l=2: DVE window reduce
                nc.vector.reduce_sum(
                    zpre_r[:, :, :, 2], ysq_r[:, :, :, 4:9],
                    axis=mybir.AxisListType.X)
                # l=3: DVE window reduce
                nc.vector.reduce_sum(
                    zpre_r[:, :, :, 3], ysq_r[:, :, :, 9:16],
                    axis=mybir.AxisListType.X)

                # --- sqrt once on ACT, then copy each half into the
                # block-diag zsb2[(h,c), (r,l,v)] (off-block zeros persist
                # in the pinned tiles, so each rl slice is a [128, 128]
                # single-free-dim lhsT with K=(h,c)) ------------------------
                zsb = zsbp.tile([128, 512], BF16, tag="zsb")
                nc.scalar.activation(zsb[:], zpre[:], AF.Sqrt)
                zsb2 = zsb2s[sg % 3]
                zsb_h = zsb[:, :].rearrange("p (a j r l) -> p a j r l",
                                            a=16, j=4, r=2)
                zsb2_h = zsb2[:, :].rearrange(
                    "p (r l v2 a j) -> p a j r l v2", r=2, l=4, v2=2, a=16)
                nc.vector.tensor_copy(zsb2_h[0:64, :, :, :, :, 0],
                                      zsb_h[0:64])
                nc.gpsimd.tensor_copy(zsb2_h[64:128, :, :, :, :, 1],
                                      zsb_h[64:128])

                # --- MM3: contract (h, c) x 8 rl against duplicated W -----
                ps3 = ps3p.tile([128, COUT], F32, tag="ps3")
                for rl in range(8):
                    nc.tensor.matmul(
                        ps3[:, :],
                        zsb2[:, COUT * rl:COUT * (rl + 1)],
                        w_t[:, COUT * rl:COUT * (rl + 1)],
                        start=(rl == 0), stop=(rl == 7),
                        skip_group_check=True)

                # --- bias + relu; store deferred one sg -------------------
                osb = osbp.tile([128, COUT], F32, tag="osb")
                if with_bias:
                    nc.vector.tensor_add(
                        osb[:], ps3[:],
                        bias_t[:, :].broadcast(0, 128))
                    nc.vector.tensor_scalar_max(osb[:], osb[:], 0.0)
                else:
                    nc.gpsimd.tensor_scalar_max(osb[:], ps3[:], 0.0)
                prev_store = (outd.ap()[VSG * sg:VSG * (sg + 1), :], osb[:])
            nc.gpsimd.dma_start(*prev_store)

    nc.compile()
    return nc


def _prep_inputs_core(b, signal, patches_idx, conv_kernel, kernel_weights,
                      biases, nsg):
    bf = mybir.dt.np(BF16)
    ngrp = nsg // KGRP
    # kre[grp, j, p, (sq, g, rn)] = conv_kernel[b, (grp*KGRP+sq)*128+4g+j, p, rn]
    k = conv_kernel[b].reshape(ngrp, KGRP, NCHUNK, 4, P, R * NSH)
    kre = np.ascontiguousarray(
        k.transpose(0, 3, 4, 1, 2, 5)).reshape(
        ngrp, 128, KGRP * NCHUNK * R * NSH).astype(bf)
    # wsb[c + 64*dup, rl*128 + i] = kernel_weights[i, c, r, l], rl = 4r + l
    w = kernel_weights.transpose(2, 3, 1, 0).reshape(8, CIN, COUT)
    wrow = np.ascontiguousarray(w.transpose(1, 0, 2)).reshape(CIN, 8 * COUT)
    wsb = np.concatenate([wrow, wrow], axis=0).astype(bf)
    bia = biases.reshape(1, COUT).astype(np.float32)
    out = {"kre": kre, "wsb": wsb, "bia": bia}
    if PREGATHER:
        # pp[grp, 32j+p, (sq, g, c)] = signal[b][pidx[v=(grp*KGRP+sq)*128+4g+j, p]]
        pb = signal[b].astype(bf)[patches_idx[b, :, :, 1]]   # [V, P, C] bf16
        pb = pb.reshape(ngrp, KGRP, NCHUNK, 4, P, CIN)
        out["pp"] = np.ascontiguousarray(
            pb.transpose(0, 3, 4, 1, 2, 5)).reshape(
            ngrp, 128, KGRP * NCHUNK * CIN)
    else:
        sig = np.zeros((V, 128), dtype=bf)
        sig[:, :CIN] = signal[b].astype(bf)
        out["sig"] = sig
        out["idx"] = _fix_idx_wrap(
            patches_idx[b, :, :, 1].astype(np.int16).reshape(-1))
    return out


def _fix_idx_wrap(pidx_flat):
    # wrap order is per gather op: each op's GSZ idxs wrapped into 16
    # partitions independently.
    blk = pidx_flat.reshape(-1, GSZ // 16, 16)   # [ops, GSZ/16, 16]
    out = np.ascontiguousarray(
        blk.transpose(0, 2, 1).transpose(1, 0, 2)).reshape(16, -1)
    return np.tile(out, (8, 1))


def _make_runner(nc, n_cores=8):
    import jax
    from jax.sharding import Mesh, PartitionSpec
    from jax.experimental.shard_map import shard_map

    bass2jax.install_neuronx_cc_hook()
    partition_name = (nc.partition_id_tensor.name
                      if nc.partition_id_tensor else None)
    in_names, out_names, out_avals, zero_outs = [], [], [], []
    for alloc in nc.m.functions[0].allocations:
        if not isinstance(alloc, mybir.MemoryLocationSet):
            continue
        name = alloc.memorylocations[0].name
        if alloc.kind == "ExternalInput":
            if name != partition_name:
                in_names.append(name)
        elif alloc.kind == "ExternalOutput":
            out_names.append(name)
            shape = tuple(alloc.tensor_shape)
            dtype = mybir.dt.np(alloc.dtype)
            out_avals.append(jax.core.ShapedArray(shape, dtype))
            zero_outs.append(np.zeros(shape, dtype))
    n_params, n_outs = len(in_names), len(out_avals)
    in_names_all = list(in_names) + list(out_names)
    if partition_name is not None:
        in_names_all.append(partition_name)

    def _body(*args):
        operands = list(args)
        if partition_name is not None:
            operands.append(bass2jax.partition_id_tensor())
        outs = bass2jax._bass_exec_p.bind(
            *operands, out_avals=tuple(out_avals),
            in_names=tuple(in_names_all), out_names=tuple(out_names),
            lowering_input_output_aliases=(),
            sim_require_finite=True, sim_require_nnan=True, nc=nc)
        return tuple(outs)

    donate = tuple(range(n_params, n_params + n_outs))
    devices = jax.devices()[:n_cores]
    mesh = Mesh(np.asarray(devices), ("core",))
    sharded = jax.jit(
        shard_map(_body, mesh=mesh,
                  in_specs=(PartitionSpec("core"),) * (n_params + n_outs),
                  out_specs=(PartitionSpec("core"),) * n_outs,
                  check_rep=False),
        donate_argnums=donate, keep_unused=True)

    def run_fn(in_maps):
        import jax
        per_core = [[np.asarray(m[nm]) for nm in in_names] for m in in_maps]
        concat_in = [
            np.concatenate([per_core[c][i] for c in range(n_cores)], axis=0)
            for i in range(n_params)]
        concat_zeros = [
            np.zeros((n_cores * z.shape[0], *z.shape[1:]), z.dtype)
            for z in zero_outs]
        out_arrs = sharded(*concat_in, *concat_zeros)
        jax.block_until_ready(out_arrs)
        return [
            {nm: np.asarray(out_arrs[i]).reshape(n_cores, *out_avals[i].shape)[c]
             for i, nm in enumerate(out_names)}
            for c in range(n_cores)]

    return run_fn


def kernel(signal, patches_idx, conv_kernel, kernel_weights, biases):
    with_bias = bool(np.any(biases))
    key = ("k", NSG, with_bias)
    if key not in _CACHE:
        nc = _build_nc(NSG, with_bias)
        _CACHE[key] = (nc, _make_runner(nc))
    nc, run = _CACHE[key]

    in_maps = []
    for b in range(B):
        m = _prep_inputs_core(b, signal, patches_idx, conv_kernel,
                              kernel_weights, biases, NSG)
        in_maps.append(m)

    results = run(in_maps)
    out = np.stack([results[b]["outd"] for b in range(B)], axis=0)
    return out.astype(np.float32)


# revision 20
# speedup vs baseline: 1.1726x; 1.0068x over previous
"""Trainium2 Bass kernel for nn_BinaryTreeShInvariantConv.

Per (b, v): gather P=32 neighbor rows of signal[b] (Cin=64), contract over P
against conv_kernel[b,v] -> y[Cin, R*N], square, sum SH orders per degree l,
sqrt(+eps), contract [Cin*R*(L+1)=512] against kernel_weights -> [Cout=128],
bias + relu.

Sharding: data-parallel over batch B=8 -> one batch per NeuronCore (SPMD).

Dataflow per core (one batch, V=4096), in supergroups of 128 v's:
  - patches arrive either via on-device dma_gather (PREGATHER=False) or as a
    host-pregathered dense stream pp[pair, (j,p), (sq, g, c)] (PREGATHER=True;
    same HBM bytes, but dense 8KB descriptors instead of random 128B rows,
    which halves the DMA-engine time and frees the GPSIMD queue).
  - Kbd: block-diagonal conv_kernel [128 (j,p), (j' 4, sq 2, g 32, rn 32)];
    off-diagonal zeros memset once and persistent; diagonal refilled every
    KGRP supergroups by 4 contiguous [32, 4KB] DMAs.
  - MM1 per 4-v chunk g (h=g//16, bank=(g//8)%2, blk=g%8): stationary
    lhsT = patches chunk [128, 64 c], moving rhs = kbd strided slice
    [128, (j' 4, rn 32)] -> psum ps1[bank][64h:, 128blk:] = [64 c, 128 (j,rn)].
    This (h, bank, blk) split makes ps3 partitions linear in v (single store).
  - square (ACT x2 banks) -> ysq bf16 [128 (h,c), (bank, blk, j, rn) 2048].
  - degree sums over n-windows: l=2,3 DVE reduce_sum; l=1 Pool adds;
    l=0 Pool copy -> zpre f32 sbuf [128, (a, b, l) 512].
  - sqrt(x + 1e-4) on ACT -> zsb bf16 (x >= 0 so eps-add ~= max(x, eps)).
  - MM3 per (h, rl): lhsT = zsb slice [64 c, 64 (a,j)], rhs = W [64, 128 i],
    accumulate 8 rl in psum -> ps3 [(h,bank,blk,j)=v 128, 128 i].
  - relu (+ bias if nonzero) on DVE -> osb; one [128, 512B-row] store per sg.
"""

import sys

sys.path.insert(0, "/opt/trn_rl_repo")

import numpy as np

import concourse.bacc as bacc
import concourse.mybir as mybir
import concourse.tile as tile
from concourse import bass2jax

B, V, P, CIN, R, COUT = 8, 4096, 32, 64, 2, 128
NSH, NDEG = 16, 4
VSG = 128            # v's per supergroup
NSG = V // VSG       # 32 supergroups
NCHUNK = VSG // 4    # 32 chunks of 4 v's
SGI = VSG * P        # 4096 gather indices per supergroup
GSZ = 4096           # indices per dma_gather op
KGRP = 4             # supergroups per kbd/patch tile refill
PREGATHER = True     # host-side signal[pidx]: dense stream vs random gather
BF16 = mybir.dt.bfloat16
F32 = mybir.dt.float32
I16 = mybir.dt.int16

_CACHE = {}


def _dma_gather_any(eng, out_ap, in_ap, idxs_ap, num_idxs, elem_size,
                    single_packet=True):
    """bass.dma_gather minus the elem_size%256 assert (the Q7 ucode only
    requires the source ROW STRIDE to be a 256B multiple; the bytes read per
    row are free). in_ap's outer stride (elem_step) must be 256B-aligned."""
    from concourse import ap_utils
    from concourse.bass import MemorySpace

    assert idxs_ap.dtype == I16
    assert in_ap.space == MemorySpace.DRAM
    assert in_ap.dtype == out_ap.dtype
    elem_step = in_ap.ap[0][0]
    stride_bytes = elem_step * mybir.dt.size(in_ap.dtype)
    assert stride_bytes % 256 == 0 and stride_bytes // 256 < 256
    assert ap_utils.ap_is_contiguous(out_ap.ap[1:])
    assert ap_utils.ap_is_contiguous(idxs_ap.ap[1:])
    assert in_ap.ap[-1][1] == out_ap.ap[-1][1] == elem_size
    assert out_ap.ap[0][1] * out_ap.ap[1][1] == ((num_idxs + 127) // 128) * 128

    _in_ap = eng.lower_ap_dma(in_ap, for_custom_bir_dma=True)
    return eng.add_instruction(
        mybir.InstDMAGatherAnt(
            name=eng.bass.get_next_instruction_name(),
            ins=[*_in_ap, eng.lower_ap(idxs_ap),
                 eng.lower_val_access(eng.to_reg(num_idxs))],
            outs=[eng.lower_ap(out_ap)],
            transpose=False,
            num_idxs=num_idxs,
            elem_size=elem_size,
            stride_bytes_256=stride_bytes // 256,
            gen_mode=0,
            single_packet=single_packet,
            queue_num=0,
            sbuf_tokens_per_rank=0,
            sbuf_free_dim_per_rank=0,
            sbuf_free_dim_pad_per_rank=0,
            sbuf_byte_offset=0,
        ))


def _build_nc(nsg, with_bias):
    nc = bacc.Bacc("TRN2", target_bir_lowering=False, debug=False,
                   enable_asserts=False)
    vtot = nsg * VSG
    ngrp = nsg // KGRP
    kcols = KGRP * NCHUNK * R * NSH          # cols per j block: (sq, g, rn)
    if PREGATHER:
        pp = nc.dram_tensor("pp", [ngrp, 128, KGRP * NCHUNK * CIN], BF16,
                            kind="ExternalInput")
    else:
        sig = nc.dram_tensor("sig", [V, 128], BF16, kind="ExternalInput")
        idx = nc.dram_tensor("idx", [128, (SGI // 16) * nsg], I16,
                             kind="ExternalInput")
    kre = nc.dram_tensor("kre", [ngrp, 128, kcols], BF16,
                         kind="ExternalInput")
    wsb = nc.dram_tensor("wsb", [128, 8 * COUT], BF16, kind="ExternalInput")
    bia = nc.dram_tensor("bia", [1, COUT], F32, kind="ExternalInput")
    outd = nc.dram_tensor("outd", [vtot, COUT], F32, kind="ExternalOutput")

    AF = mybir.ActivationFunctionType
    ALU = mybir.AluOpType
    with tile.TileContext(nc) as tc:
        with (
            tc.tile_pool(name="const", bufs=1) as constp,
            tc.tile_pool(name="kbd", bufs=3) as stgp,
            tc.tile_pool(name="patches", bufs=3) as patp,
            tc.tile_pool(name="ysq", bufs=2) as ysqp,
            tc.tile_pool(name="zpre", bufs=2) as zprep,
            tc.tile_pool(name="zsb", bufs=3) as zsbp,
            tc.tile_pool(name="ytmp", bufs=3) as ytmpp,
            tc.tile_pool(name="osb", bufs=3) as osbp,
            tc.tile_pool(name="ps1", bufs=3, space="PSUM") as ps1p,
            tc.tile_pool(name="ps3", bufs=2, space="PSUM") as ps3p,
        ):
            w_t = constp.tile([128, 8 * COUT], BF16, tag="w")
            nc.sync.dma_start(w_t[:], wsb.ap())
            if not PREGATHER:
                idx_t = constp.tile([128, (SGI // 16) * nsg], I16, tag="idx")
                nc.sync.dma_start(idx_t[:], idx.ap())
            if with_bias:
                bias_t = constp.tile([1, COUT], F32, tag="bias")
                nc.sync.dma_start(bias_t[:], bia.ap())

            eps_t = constp.tile([128, 1], F32, tag="eps")
            nc.vector.memset(eps_t[:], 1e-4)

            stg_tiles = {}

            zsb2s = [zsbp.tile([128, 8 * COUT], BF16, tag="zsb2",
                               name=f"zsb2_{i}") for i in range(3)]
            for i in range(3):
                nc.vector.memset(zsb2s[i][:], 0.0)

            pat_tiles = {}

            def issue_pat(g, split=False):
                t = patp.tile([128, KGRP, NCHUNK, CIN], BF16, tag="pat",
                              name=f"pat{g % 3}")
                pat_tiles[g] = t
                if split:
                    nc.sync.dma_start(t[:, 0], pp.ap()[g].rearrange(
                        "p (sq r) -> p sq r", sq=KGRP)[:, 0])
                    nc.sync.dma_start(
                        t[:, 1:], pp.ap()[g].rearrange(
                            "p (sq r) -> p sq r", sq=KGRP)[:, 1:])
                else:
                    nc.sync.dma_start(t[:], pp.ap()[g])

            def issue_kbd(g):
                t = stgp.tile([128, kcols], BF16, tag="stg",
                              name=f"stg{g % 3}")
                stg_tiles[g] = t
                nc.sync.dma_start(t[:], kre.ap()[g])

            if PREGATHER:
                issue_pat(0, split=True)
                issue_pat(1)
            issue_kbd(0)
            issue_kbd(1)

            prev_store = None
            for sg in range(nsg):
                grp, sq = sg // KGRP, sg % KGRP
                if sq == 0 and grp + 2 < ngrp:
                    issue_kbd(grp + 2)
                stg_r = stg_tiles[grp][:, :].rearrange(
                    "p (sq g rn) -> p sq g rn", sq=KGRP, g=NCHUNK)

                # --- patches ----------------------------------------------
                if PREGATHER:
                    if sq == 0 and grp + 2 < ngrp:
                        issue_pat(grp + 2)
                    pat = pat_tiles[grp][:, sq]
                else:
                    patg = patp.tile([128, NCHUNK, CIN], BF16, tag="pat")
                    pat_r = patg[:, :, :].rearrange("p (t u) c -> p t u c",
                                                    t=1)
                    _dma_gather_any(
                        nc.gpsimd, pat_r[:, 0], sig.ap()[:, 0:CIN],
                        idx_t[:, (SGI // 16) * sg:(SGI // 16) * (sg + 1)],
                        GSZ, CIN, single_packet=False)
                    pat = patg[:, :, :]

                # store for the previous sg (data long ready; avoids
                # blocking any queue on this sg's compute chain)
                if prev_store is not None:
                    nc.gpsimd.dma_start(*prev_store)
                    prev_store = None

                # --- MM1: chunk g = 16h + 8b + blk -> ps1[b][64h, 128blk] -
                ps1 = [ps1p.tile([128, 1024], F32, tag="ps1",
                                 name=f"ps1_{sg}_{b}") for b in range(2)]
                for b in range(2):
                    for h in range(2):
                        for blk in range(8):
                            g = 16 * h + 8 * b + blk
                            for j in range(4):
                                nc.tensor.matmul(
                                    ps1[b][64 * h:64 * (h + 1),
                                           128 * blk + 32 * j:
                                           128 * blk + 32 * (j + 1)],
                                    pat[32 * j:32 * (j + 1), g, :],
                                    stg_r[32 * j:32 * (j + 1), sq, g, :],
                                    start=True, stop=True,
                                    tile_position=(32 * j, 64 * h))

                # --- square: ACT takes 1.5 banks; DVE squares the last
                # half-bank via a bf16 copy (one PSUM input) + 2x-mode mult -
                ysq = ysqp.tile([128, 2048], BF16, tag="ysq")
                nc.scalar.activation(ysq[:, 0:1024], ps1[0][:], AF.Square)
                nc.scalar.activation(ysq[:, 1024:1536],
                                     ps1[1][:, 0:512], AF.Square)
                ytmp = ytmpp.tile([128, 512], BF16, tag="ytmp")
                nc.vector.tensor_copy(ytmp[:], ps1[1][:, 512:1024])
                nc.vector.tensor_mul(ysq[:, 1536:2048], ytmp[:], ytmp[:])

                # --- degree sums over n-windows ---------------------------
                # ysq free: (a=(b,blk) 16, bb=(j,r) 8, n 16)
                zpre = zprep.tile([128, 512], F32, tag="zpre")
                ysq_r = ysq[:, :].rearrange("p (a bb n) -> p a bb n",
                                            a=16, bb=8)
                zpre_r = zpre[:, :].rearrange("p (a bb l) -> p a bb l",
                                              a=16, bb=8)
                # l=0: plain copy (Pool)
                nc.gpsimd.tensor_copy(zpre_r[:, :, :, 0], ysq_r[:, :, :, 0])
                # l=1: two adds (Pool)
                nc.gpsimd.tensor_tensor(zpre_r[:, :, :, 1],
                                        ysq_r[:, :, :, 1],
                                        ysq_r[:, :, :, 2], ALU.add)
                nc.gpsimd.tensor_tensor(zpre_r[:, :, :, 1],
                                        zpre_r[:, :, :, 1],
                                        ysq_r[:, :, :, 3], ALU.add)
                # l=2: DVE window reduce
                nc.vector.reduce_sum(
                    zpre_r[:, :, :, 2], ysq_r[:, :, :, 4:9],
                    axis=mybir.AxisListType.X)
                # l=3: six adds (Pool)
                nc.gpsimd.tensor_tensor(zpre_r[:, :, :, 3],
                                        ysq_r[:, :, :, 9],
                                        ysq_r[:, :, :, 10], ALU.add)
                for n in range(11, 16):
                    nc.gpsimd.tensor_tensor(zpre_r[:, :, :, 3],
                                            zpre_r[:, :, :, 3],
                                            ysq_r[:, :, :, n], ALU.add)

                # --- sqrt once on ACT, then copy each half into the
                # block-diag zsb2[(h,c), (r,l,v)] (off-block zeros persist
                # in the pinned tiles, so each rl slice is a [128, 128]
                # single-free-dim lhsT with K=(h,c)) ------------------------
                zsb = zsbp.tile([128, 512], BF16, tag="zsb")
                nc.scalar.activation(zsb[:], zpre[:], AF.Sqrt)
                zsb2 = zsb2s[sg % 3]
                zsb_h = zsb[:, :].rearrange("p (a j r l) -> p a j r l",
                                            a=16, j=4, r=2)
                zsb2_h = zsb2[:, :].rearrange(
                    "p (r l v2 a j) -> p a j r l v2", r=2, l=4, v2=2, a=16)
                nc.vector.tensor_copy(zsb2_h[0:64, :, :, :, :, 0],
                                      zsb_h[0:64])
                nc.gpsimd.tensor_copy(zsb2_h[64:128, :, :, :, :, 1],
                                      zsb_h[64:128])

                # --- MM3: contract (h, c) x 8 rl against duplicated W -----
                ps3 = ps3p.tile([128, COUT], F32, tag="ps3")
                for rl in range(8):
                    nc.tensor.matmul(
                        ps3[:, :],
                        zsb2[:, COUT * rl:COUT * (rl + 1)],
                        w_t[:, COUT * rl:COUT * (rl + 1)],
                        start=(rl == 0), stop=(rl == 7),
                        skip_group_check=True)

                # --- bias + relu; store deferred one sg -------------------
                osb = osbp.tile([128, COUT], F32, tag="osb")
                if with_bias:
                    nc.vector.tensor_add(
                        osb[:], ps3[:],
                        bias_t[:, :].broadcast(0, 128))
                    nc.vector.tensor_scalar_max(osb[:], osb[:], 0.0)
                else:
                    nc.gpsimd.tensor_scalar_max(osb[:], ps3[:], 0.0)
                prev_store = (outd.ap()[VSG * sg:VSG * (sg + 1), :], osb[:])
            nc.gpsimd.dma_start(*prev_store)

    nc.compile()
    return nc


def _prep_inputs_core(b, signal, patches_idx, conv_kernel, kernel_weights,
                      biases, nsg):
    bf = mybir.dt.np(BF16)
    ngrp = nsg // KGRP
    # kre[grp, j, p, (sq, g, rn)] = conv_kernel[b, (grp*KGRP+sq)*128+4g+j, p, rn]
    k = conv_kernel[b].reshape(ngrp, KGRP, NCHUNK, 4, P, R * NSH)
    kre = np.ascontiguousarray(
        k.transpose(0, 3, 4, 1, 2, 5)).reshape(
        ngrp, 128, KGRP * NCHUNK * R * NSH).astype(bf)
    # wsb[c + 64*dup, rl*128 + i] = kernel_weights[i, c, r, l], rl = 4r + l
    w = kernel_weights.transpose(2, 3, 1, 0).reshape(8, CIN, COUT)
    wrow = np.ascontiguousarray(w.transpose(1, 0, 2)).reshape(CIN, 8 * COUT)
    wsb = np.concatenate([wrow, wrow], axis=0).astype(bf)
    bia = biases.reshape(1, COUT).astype(np.float32)
    out = {"kre": kre, "wsb": wsb, "bia": bia}
    if PREGATHER:
        # pp[grp, 32j+p, (sq, g, c)] = signal[b][pidx[v=(grp*KGRP+sq)*128+4g+j, p]]
        pb = signal[b].astype(bf)[patches_idx[b, :, :, 1]]   # [V, P, C] bf16
        pb = pb.reshape(ngrp, KGRP, NCHUNK, 4, P, CIN)
        out["pp"] = np.ascontiguousarray(
            pb.transpose(0, 3, 4, 1, 2, 5)).reshape(
            ngrp, 128, KGRP * NCHUNK * CIN)
    else:
        sig = np.zeros((V, 128), dtype=bf)
        sig[:, :CIN] = signal[b].astype(bf)
        out["sig"] = sig
        out["idx"] = _fix_idx_wrap(
            patches_idx[b, :, :, 1].astype(np.int16).reshape(-1))
    return out


def _fix_idx_wrap(pidx_flat):
    # wrap order is per gather op: each op's GSZ idxs wrapped into 16
    # partitions independently.
    blk = pidx_flat.reshape(-1, GSZ // 16, 16)   # [ops, GSZ/16, 16]
    out = np.ascontiguousarray(
        blk.transpose(0, 2, 1).transpose(1, 0, 2)).reshape(16, -1)
    return np.tile(out, (8, 1))


def _make_runner(nc, n_cores=8):
    import jax
    from jax.sharding import Mesh, PartitionSpec
    from jax.experimental.shard_map import shard_map

    bass2jax.install_neuronx_cc_hook()
    partition_name = (nc.partition_id_tensor.name
                      if nc.partition_id_tensor else None)
    in_names, out_names, out_avals, zero_outs = [], [], [], []
    for alloc in nc.m.functions[0].allocations:
        if not isinstance(alloc, mybir.MemoryLocationSet):
            continue
        name = alloc.memorylocations[0].name
        if alloc.kind == "ExternalInput":
            if name != partition_name:
                in_names.append(name)
        elif alloc.kind == "ExternalOutput":
            out_names.append(name)
            shape = tuple(alloc.tensor_shape)
            dtype = mybir.dt.np(alloc.dtype)
            out_avals.append(jax.core.ShapedArray(shape, dtype))
            zero_outs.append(np.zeros(shape, dtype))
    n_params, n_outs = len(in_names), len(out_avals)
    in_names_all = list(in_names) + list(out_names)
    if partition_name is not None:
        in_names_all.append(partition_name)

    def _body(*args):
        operands = list(args)
        if partition_name is not None:
            operands.append(bass2jax.partition_id_tensor())
        outs = bass2jax._bass_exec_p.bind(
            *operands, out_avals=tuple(out_avals),
            in_names=tuple(in_names_all), out_names=tuple(out_names),
            lowering_input_output_aliases=(),
            sim_require_finite=True, sim_require_nnan=True, nc=nc)
        return tuple(outs)

    donate = tuple(range(n_params, n_params + n_outs))
    devices = jax.devices()[:n_cores]
    mesh = Mesh(np.asarray(devices), ("core",))
    sharded = jax.jit(
        shard_map(_body, mesh=mesh,
                  in_specs=(PartitionSpec("core"),) * (n_params + n_outs),
                  out_specs=(PartitionSpec("core"),) * n_outs,
                  check_rep=False),
        donate_argnums=donate, keep_unused=True)

    def run_fn(in_maps):
        import jax
        per_core = [[np.asarray(m[nm]) for nm in in_names] for m in in_maps]
        concat_in = [
            np.concatenate([per_core[c][i] for c in range(n_cores)], axis=0)
            for i in range(n_params)]
        concat_zeros = [
            np.zeros((n_cores * z.shape[0], *z.shape[1:]), z.dtype)
            for z in zero_outs]
        out_arrs = sharded(*concat_in, *concat_zeros)
        jax.block_until_ready(out_arrs)
        return [
            {nm: np.asarray(out_arrs[i]).reshape(n_cores, *out_avals[i].shape)[c]
             for i, nm in enumerate(out_names)}
            for c in range(n_cores)]

    return run_fn


def kernel(signal, patches_idx, conv_kernel, kernel_weights, biases):
    with_bias = bool(np.any(biases))
    key = ("k", NSG, with_bias)
    if key not in _CACHE:
        nc = _build_nc(NSG, with_bias)
        _CACHE[key] = (nc, _make_runner(nc))
    nc, run = _CACHE[key]

    in_maps = []
    for b in range(B):
        m = _prep_inputs_core(b, signal, patches_idx, conv_kernel,
                              kernel_weights, biases, NSG)
        in_maps.append(m)

    results = run(in_maps)
    out = np.stack([results[b]["outd"] for b in range(B)], axis=0)
    return out.astype(np.float32)


# revision 22
# speedup vs baseline: 1.2420x; 1.0592x over previous
"""Trainium2 Bass kernel for nn_BinaryTreeShInvariantConv.

Per (b, v): gather P=32 neighbor rows of signal[b] (Cin=64), contract over P
against conv_kernel[b,v] -> y[Cin, R*N], square, sum SH orders per degree l,
sqrt(+eps), contract [Cin*R*(L+1)=512] against kernel_weights -> [Cout=128],
bias + relu.

Sharding: data-parallel over batch B=8 -> one batch per NeuronCore (SPMD).

Dataflow per core (one batch, V=4096), in supergroups of 128 v's:
  - patches arrive either via on-device dma_gather (PREGATHER=False) or as a
    host-pregathered dense stream pp[pair, (j,p), (sq, g, c)] (PREGATHER=True;
    same HBM bytes, but dense 8KB descriptors instead of random 128B rows,
    which halves the DMA-engine time and frees the GPSIMD queue).
  - Kbd: block-diagonal conv_kernel [128 (j,p), (j' 4, sq 2, g 32, rn 32)];
    off-diagonal zeros memset once and persistent; diagonal refilled every
    KGRP supergroups by 4 contiguous [32, 4KB] DMAs.
  - MM1 per 4-v chunk g (h=g//16, bank=(g//8)%2, blk=g%8): stationary
    lhsT = patches chunk [128, 64 c], moving rhs = kbd strided slice
    [128, (j' 4, rn 32)] -> psum ps1[bank][64h:, 128blk:] = [64 c, 128 (j,rn)].
    This (h, bank, blk) split makes ps3 partitions linear in v (single store).
  - square (ACT x2 banks) -> ysq bf16 [128 (h,c), (bank, blk, j, rn) 2048].
  - degree sums over n-windows: l=2,3 DVE reduce_sum; l=1 Pool adds;
    l=0 Pool copy -> zpre f32 sbuf [128, (a, b, l) 512].
  - sqrt(x + 1e-4) on ACT -> zsb bf16 (x >= 0 so eps-add ~= max(x, eps)).
  - MM3 per (h, rl): lhsT = zsb slice [64 c, 64 (a,j)], rhs = W [64, 128 i],
    accumulate 8 rl in psum -> ps3 [(h,bank,blk,j)=v 128, 128 i].
  - relu (+ bias if nonzero) on DVE -> osb; one [128, 512B-row] store per sg.
"""

import sys

sys.path.insert(0, "/opt/trn_rl_repo")

import numpy as np

import concourse.bacc as bacc
import concourse.mybir as mybir
import concourse.tile as tile
from concourse import bass2jax

B, V, P, CIN, R, COUT = 8, 4096, 32, 64, 2, 128
NSH, NDEG = 16, 4
VSG = 128            # v's per supergroup
NSG = V // VSG       # 32 supergroups
NCHUNK = VSG // 4    # 32 chunks of 4 v's
SGI = VSG * P        # 4096 gather indices per supergroup
GSZ = 4096           # indices per dma_gather op
KGRP = 4             # supergroups per kbd/patch tile refill
PREGATHER = True     # host-side signal[pidx]: dense stream vs random gather
BF16 = mybir.dt.bfloat16
F32 = mybir.dt.float32
I16 = mybir.dt.int16

_CACHE = {}


def _dma_gather_any(eng, out_ap, in_ap, idxs_ap, num_idxs, elem_size,
                    single_packet=True):
    """bass.dma_gather minus the elem_size%256 assert (the Q7 ucode only
    requires the source ROW STRIDE to be a 256B multiple; the bytes read per
    row are free). in_ap's outer stride (elem_step) must be 256B-aligned."""
    from concourse import ap_utils
    from concourse.bass import MemorySpace

    assert idxs_ap.dtype == I16
    assert in_ap.space == MemorySpace.DRAM
    assert in_ap.dtype == out_ap.dtype
    elem_step = in_ap.ap[0][0]
    stride_bytes = elem_step * mybir.dt.size(in_ap.dtype)
    assert stride_bytes % 256 == 0 and stride_bytes // 256 < 256
    assert ap_utils.ap_is_contiguous(out_ap.ap[1:])
    assert ap_utils.ap_is_contiguous(idxs_ap.ap[1:])
    assert in_ap.ap[-1][1] == out_ap.ap[-1][1] == elem_size
    assert out_ap.ap[0][1] * out_ap.ap[1][1] == ((num_idxs + 127) // 128) * 128

    _in_ap = eng.lower_ap_dma(in_ap, for_custom_bir_dma=True)
    return eng.add_instruction(
        mybir.InstDMAGatherAnt(
            name=eng.bass.get_next_instruction_name(),
            ins=[*_in_ap, eng.lower_ap(idxs_ap),
                 eng.lower_val_access(eng.to_reg(num_idxs))],
            outs=[eng.lower_ap(out_ap)],
            transpose=False,
            num_idxs=num_idxs,
            elem_size=elem_size,
            stride_bytes_256=stride_bytes // 256,
            gen_mode=0,
            single_packet=single_packet,
            queue_num=0,
            sbuf_tokens_per_rank=0,
            sbuf_free_dim_per_rank=0,
            sbuf_free_dim_pad_per_rank=0,
            sbuf_byte_offset=0,
        ))


def _build_nc(nsg, with_bias):
    nc = bacc.Bacc("TRN2", target_bir_lowering=False, debug=False,
                   enable_asserts=False)
    vtot = nsg * VSG
    ngrp = nsg // KGRP
    kcols = KGRP * NCHUNK * R * NSH          # cols per j block: (sq, g, rn)
    if PREGATHER:
        pp = nc.dram_tensor("pp", [ngrp, 128, KGRP * NCHUNK * CIN], BF16,
                            kind="ExternalInput")
    else:
        sig = nc.dram_tensor("sig", [V, 128], BF16, kind="ExternalInput")
        idx = nc.dram_tensor("idx", [128, (SGI // 16) * nsg], I16,
                             kind="ExternalInput")
    kre = nc.dram_tensor("kre", [ngrp, 128, kcols], BF16,
                         kind="ExternalInput")
    wsb = nc.dram_tensor("wsb", [128, 8 * COUT], BF16, kind="ExternalInput")
    bia = nc.dram_tensor("bia", [1, COUT], F32, kind="ExternalInput")
    outd = nc.dram_tensor("outd", [vtot, COUT], F32, kind="ExternalOutput")

    AF = mybir.ActivationFunctionType
    ALU = mybir.AluOpType
    with tile.TileContext(nc) as tc:
        with (
            tc.tile_pool(name="const", bufs=1) as constp,
            tc.tile_pool(name="kbd", bufs=3) as stgp,
            tc.tile_pool(name="patches", bufs=3) as patp,
            tc.tile_pool(name="ysq", bufs=2) as ysqp,
            tc.tile_pool(name="zpre", bufs=2) as zprep,
            tc.tile_pool(name="zsb", bufs=3) as zsbp,
            tc.tile_pool(name="ytmp", bufs=3) as ytmpp,
            tc.tile_pool(name="osb", bufs=3) as osbp,
            tc.tile_pool(name="ps1", bufs=3, space="PSUM") as ps1p,
            tc.tile_pool(name="ps3", bufs=2, space="PSUM") as ps3p,
        ):
            w_t = constp.tile([128, 8 * COUT], BF16, tag="w")
            nc.sync.dma_start(w_t[:], wsb.ap())
            if not PREGATHER:
                idx_t = constp.tile([128, (SGI // 16) * nsg], I16, tag="idx")
                nc.sync.dma_start(idx_t[:], idx.ap())
            if with_bias:
                bias_t = constp.tile([1, COUT], F32, tag="bias")
                nc.sync.dma_start(bias_t[:], bia.ap())

            eps_t = constp.tile([128, 1], F32, tag="eps")
            nc.vector.memset(eps_t[:], 1e-4)

            stg_tiles = {}

            zsb2s = [zsbp.tile([128, 8 * COUT], BF16, tag="zsb2",
                               name=f"zsb2_{i}") for i in range(3)]
            for i in range(3):
                nc.vector.memset(zsb2s[i][:], 0.0)

            pat_tiles = {}

            def issue_pat(g, split=False):
                t = patp.tile([128, KGRP, NCHUNK, CIN], BF16, tag="pat",
                              name=f"pat{g % 3}")
                pat_tiles[g] = t
                if split:
                    for s in range(KGRP):
                        nc.sync.dma_start(t[:, s], pp.ap()[g].rearrange(
                            "p (sq r) -> p sq r", sq=KGRP)[:, s])
                else:
                    nc.sync.dma_start(t[:], pp.ap()[g])

            def issue_kbd(g):
                t = stgp.tile([128, kcols], BF16, tag="stg",
                              name=f"stg{g % 3}")
                stg_tiles[g] = t
                nc.sync.dma_start(t[:], kre.ap()[g])

            if PREGATHER:
                issue_pat(0, split=True)
                issue_kbd(0)
                issue_pat(1)
                issue_kbd(1)
            else:
                issue_kbd(0)
                issue_kbd(1)

            prev_store = None
            for sg in range(nsg):
                grp, sq = sg // KGRP, sg % KGRP
                if sq == 0 and grp + 2 < ngrp:
                    issue_kbd(grp + 2)
                stg_r = stg_tiles[grp][:, :].rearrange(
                    "p (sq g rn) -> p sq g rn", sq=KGRP, g=NCHUNK)

                # --- patches ----------------------------------------------
                if PREGATHER:
                    if sq == 0 and grp + 2 < ngrp:
                        issue_pat(grp + 2)
                    pat = pat_tiles[grp][:, sq]
                else:
                    patg = patp.tile([128, NCHUNK, CIN], BF16, tag="pat")
                    pat_r = patg[:, :, :].rearrange("p (t u) c -> p t u c",
                                                    t=1)
                    _dma_gather_any(
                        nc.gpsimd, pat_r[:, 0], sig.ap()[:, 0:CIN],
                        idx_t[:, (SGI // 16) * sg:(SGI // 16) * (sg + 1)],
                        GSZ, CIN, single_packet=False)
                    pat = patg[:, :, :]

                # store for the previous sg (data long ready; avoids
                # blocking any queue on this sg's compute chain)
                if prev_store is not None:
                    nc.gpsimd.dma_start(*prev_store)
                    prev_store = None

                # --- MM1: chunk g = 16h + 8b + blk -> ps1[b][64h, 128blk] -
                ps1 = [ps1p.tile([128, 1024], F32, tag="ps1",
                                 name=f"ps1_{sg}_{b}") for b in range(2)]
                for b in range(2):
                    for h in range(2):
                        for blk in range(8):
                            g = 16 * h + 8 * b + blk
                            for j in range(4):
                                nc.tensor.matmul(
                                    ps1[b][64 * h:64 * (h + 1),
                                           128 * blk + 32 * j:
                                           128 * blk + 32 * (j + 1)],
                                    pat[32 * j:32 * (j + 1), g, :],
                                    stg_r[32 * j:32 * (j + 1), sq, g, :],
                                    start=True, stop=True,
                                    tile_position=(32 * j, 64 * h))

                # --- square: ACT takes 1.5 banks; DVE squares the last
                # half-bank via a bf16 copy (one PSUM input) + 2x-mode mult -
                ysq = ysqp.tile([128, 2048], BF16, tag="ysq")
                nc.scalar.activation(ysq[:, 0:1024], ps1[0][:], AF.Square)
                nc.scalar.activation(ysq[:, 1024:1536],
                                     ps1[1][:, 0:512], AF.Square)
                ytmp = ytmpp.tile([128, 512], BF16, tag="ytmp")
                nc.vector.tensor_copy(ytmp[:], ps1[1][:, 512:1024])
                nc.vector.tensor_mul(ysq[:, 1536:2048], ytmp[:], ytmp[:])

                # --- degree sums over n-windows ---------------------------
                # ysq free: (a=(b,blk) 16, bb=(j,r) 8, n 16)
                zpre = zprep.tile([128, 512], F32, tag="zpre")
                ysq_r = ysq[:, :].rearrange("p (a bb n) -> p a bb n",
                                            a=16, bb=8)
                zpre_r = zpre[:, :].rearrange("p (a bb l) -> p a bb l",
                                              a=16, bb=8)
                # l=0: plain copy (Pool)
                nc.gpsimd.tensor_copy(zpre_r[:, :, :, 0], ysq_r[:, :, :, 0])
                # l=1: two adds (Pool)
                nc.gpsimd.tensor_tensor(zpre_r[:, :, :, 1],
                                        ysq_r[:, :, :, 1],
                                        ysq_r[:, :, :, 2], ALU.add)
                nc.gpsimd.tensor_tensor(zpre_r[:, :, :, 1],
                                        zpre_r[:, :, :, 1],
                                        ysq_r[:, :, :, 3], ALU.add)
                # l=2: DVE window reduce
                nc.vector.reduce_sum(
                    zpre_r[:, :, :, 2], ysq_r[:, :, :, 4:9],
                    axis=mybir.AxisListType.X)
                # l=3: six adds (Pool)
                nc.gpsimd.tensor_tensor(zpre_r[:, :, :, 3],
                                        ysq_r[:, :, :, 9],
                                        ysq_r[:, :, :, 10], ALU.add)
                for n in range(11, 16):
                    nc.gpsimd.tensor_tensor(zpre_r[:, :, :, 3],
                                            zpre_r[:, :, :, 3],
                                            ysq_r[:, :, :, n], ALU.add)

                # --- sqrt once on ACT, then copy each half into the
                # block-diag zsb2[(h,c), (r,l,v)] (off-block zeros persist
                # in the pinned tiles, so each rl slice is a [128, 128]
                # single-free-dim lhsT with K=(h,c)) ------------------------
                zsb = zsbp.tile([128, 512], BF16, tag="zsb")
                nc.scalar.activation(zsb[:], zpre[:], AF.Sqrt)
                zsb2 = zsb2s[sg % 3]
                zsb_h = zsb[:, :].rearrange("p (a j r l) -> p a j r l",
                                            a=16, j=4, r=2)
                zsb2_h = zsb2[:, :].rearrange(
                    "p (r l v2 a j) -> p a j r l v2", r=2, l=4, v2=2, a=16)
                nc.vector.tensor_copy(zsb2_h[0:64, :, :, :, :, 0],
                                      zsb_h[0:64])
                nc.gpsimd.tensor_copy(zsb2_h[64:128, :, :, :, :, 1],
                                      zsb_h[64:128])

                # --- MM3: contract (h, c) x 8 rl against duplicated W -----
                ps3 = ps3p.tile([128, COUT], F32, tag="ps3")
                for rl in range(8):
                    nc.tensor.matmul(
                        ps3[:, :],
                        zsb2[:, COUT * rl:COUT * (rl + 1)],
                        w_t[:, COUT * rl:COUT * (rl + 1)],
                        start=(rl == 0), stop=(rl == 7),
                        skip_group_check=True)

                # --- bias + relu; store deferred one sg -------------------
                osb = osbp.tile([128, COUT], F32, tag="osb")
                if with_bias:
                    nc.vector.tensor_add(
                        osb[:], ps3[:],
                        bias_t[:, :].broadcast(0, 128))
                    nc.vector.tensor_scalar_max(osb[:], osb[:], 0.0)
                else:
                    nc.gpsimd.tensor_scalar_max(osb[:], ps3[:], 0.0)
                prev_store = (outd.ap()[VSG * sg:VSG * (sg + 1), :], osb[:])
            nc.gpsimd.dma_start(*prev_store)

    nc.compile()
    return nc


def _prep_inputs_core(b, signal, patches_idx, conv_kernel, kernel_weights,
                      biases, nsg):
    bf = mybir.dt.np(BF16)
    ngrp = nsg // KGRP
    # kre[grp, j, p, (sq, g, rn)] = conv_kernel[b, (grp*KGRP+sq)*128+4g+j, p, rn]
    k = conv_kernel[b].reshape(ngrp, KGRP, NCHUNK, 4, P, R * NSH)
    kre = np.ascontiguousarray(
        k.transpose(0, 3, 4, 1, 2, 5)).reshape(
        ngrp, 128, KGRP * NCHUNK * R * NSH).astype(bf)
    # wsb[c + 64*dup, rl*128 + i] = kernel_weights[i, c, r, l], rl = 4r + l
    w = kernel_weights.transpose(2, 3, 1, 0).reshape(8, CIN, COUT)
    wrow = np.ascontiguousarray(w.transpose(1, 0, 2)).reshape(CIN, 8 * COUT)
    wsb = np.concatenate([wrow, wrow], axis=0).astype(bf)
    bia = biases.reshape(1, COUT).astype(np.float32)
    out = {"kre": kre, "wsb": wsb, "bia": bia}
    if PREGATHER:
        # pp[grp, 32j+p, (sq, g, c)] = signal[b][pidx[v=(grp*KGRP+sq)*128+4g+j, p]]
        pb = signal[b].astype(bf)[patches_idx[b, :, :, 1]]   # [V, P, C] bf16
        pb = pb.reshape(ngrp, KGRP, NCHUNK, 4, P, CIN)
        out["pp"] = np.ascontiguousarray(
            pb.transpose(0, 3, 4, 1, 2, 5)).reshape(
            ngrp, 128, KGRP * NCHUNK * CIN)
    else:
        sig = np.zeros((V, 128), dtype=bf)
        sig[:, :CIN] = signal[b].astype(bf)
        out["sig"] = sig
        out["idx"] = _fix_idx_wrap(
            patches_idx[b, :, :, 1].astype(np.int16).reshape(-1))
    return out


def _fix_idx_wrap(pidx_flat):
    # wrap order is per gather op: each op's GSZ idxs wrapped into 16
    # partitions independently.
    blk = pidx_flat.reshape(-1, GSZ // 16, 16)   # [ops, GSZ/16, 16]
    out = np.ascontiguousarray(
        blk.transpose(0, 2, 1).transpose(1, 0, 2)).reshape(16, -1)
    return np.tile(out, (8, 1))


def _make_runner(nc, n_cores=8):
    import jax
    from jax.sharding import Mesh, PartitionSpec
    from jax.experimental.shard_map import shard_map

    bass2jax.install_neuronx_cc_hook()
    partition_name = (nc.partition_id_tensor.name
                      if nc.partition_id_tensor else None)
    in_names, out_names, out_avals, zero_outs = [], [], [], []
    for alloc in nc.m.functions[0].allocations:
        if not isinstance(alloc, mybir.MemoryLocationSet):
            continue
        name = alloc.memorylocations[0].name
        if alloc.kind == "ExternalInput":
            if name != partition_name:
                in_names.append(name)
        elif alloc.kind == "ExternalOutput":
            out_names.append(name)
            shape = tuple(alloc.tensor_shape)
            dtype = mybir.dt.np(alloc.dtype)
            out_avals.append(jax.core.ShapedArray(shape, dtype))
            zero_outs.append(np.zeros(shape, dtype))
    n_params, n_outs = len(in_names), len(out_avals)
    in_names_all = list(in_names) + list(out_names)
    if partition_name is not None:
        in_names_all.append(partition_name)

    def _body(*args):
        operands = list(args)
        if partition_name is not None:
            operands.append(bass2jax.partition_id_tensor())
        outs = bass2jax._bass_exec_p.bind(
            *operands, out_avals=tuple(out_avals),
            in_names=tuple(in_names_all), out_names=tuple(out_names),
            lowering_input_output_aliases=(),
            sim_require_finite=True, sim_require_nnan=True, nc=nc)
        return tuple(outs)

    donate = tuple(range(n_params, n_params + n_outs))
    devices = jax.devices()[:n_cores]
    mesh = Mesh(np.asarray(devices), ("core",))
    sharded = jax.jit(
        shard_map(_body, mesh=mesh,
                  in_specs=(PartitionSpec("core"),) * (n_params + n_outs),
                  out_specs=(PartitionSpec("core"),) * n_outs,
                  check_rep=False),
        donate_argnums=donate, keep_unused=True)

    def run_fn(in_maps):
        import jax
        per_core = [[np.asarray(m[nm]) for nm in in_names] for m in in_maps]
        concat_in = [
            np.concatenate([per_core[c][i] for c in range(n_cores)], axis=0)
            for i in range(n_params)]
        concat_zeros = [
            np.zeros((n_cores * z.shape[0], *z.shape[1:]), z.dtype)
            for z in zero_outs]
        out_arrs = sharded(*concat_in, *concat_zeros)
        jax.block_until_ready(out_arrs)
        return [
            {nm: np.asarray(out_arrs[i]).reshape(n_cores, *out_avals[i].shape)[c]
             for i, nm in enumerate(out_names)}
            for c in range(n_cores)]

    return run_fn


def kernel(signal, patches_idx, conv_kernel, kernel_weights, biases):
    with_bias = bool(np.any(biases))
    key = ("k", NSG, with_bias)
    if key not in _CACHE:
        nc = _build_nc(NSG, with_bias)
        _CACHE[key] = (nc, _make_runner(nc))
    nc, run = _CACHE[key]

    in_maps = []
    for b in range(B):
        m = _prep_inputs_core(b, signal, patches_idx, conv_kernel,
                              kernel_weights, biases, NSG)
        in_maps.append(m)

    results = run(in_maps)
    out = np.stack([results[b]["outd"] for b in range(B)], axis=0)
    return out.astype(np.float32)


# revision 23
# speedup vs baseline: 1.3705x; 1.1035x over previous
"""Trainium2 Bass kernel for nn_BinaryTreeShInvariantConv.

Per (b, v): gather P=32 neighbor rows of signal[b] (Cin=64), contract over P
against conv_kernel[b,v] -> y[Cin, R*N], square, sum SH orders per degree l,
sqrt(+eps), contract [Cin*R*(L+1)=512] against kernel_weights -> [Cout=128],
bias + relu.

Sharding: data-parallel over batch B=8 -> one batch per NeuronCore (SPMD).

Dataflow per core (one batch, V=4096), in supergroups of 128 v's:
  - patches arrive either via on-device dma_gather (PREGATHER=False) or as a
    host-pregathered dense stream pp[pair, (j,p), (sq, g, c)] (PREGATHER=True;
    same HBM bytes, but dense 8KB descriptors instead of random 128B rows,
    which halves the DMA-engine time and frees the GPSIMD queue).
  - Kbd: block-diagonal conv_kernel [128 (j,p), (j' 4, sq 2, g 32, rn 32)];
    off-diagonal zeros memset once and persistent; diagonal refilled every
    KGRP supergroups by 4 contiguous [32, 4KB] DMAs.
  - MM1 per 4-v chunk g (h=g//16, bank=(g//8)%2, blk=g%8): stationary
    lhsT = patches chunk [128, 64 c], moving rhs = kbd strided slice
    [128, (j' 4, rn 32)] -> psum ps1[bank][64h:, 128blk:] = [64 c, 128 (j,rn)].
    This (h, bank, blk) split makes ps3 partitions linear in v (single store).
  - square (ACT x2 banks) -> ysq bf16 [128 (h,c), (bank, blk, j, rn) 2048].
  - degree sums over n-windows: l=2,3 DVE reduce_sum; l=1 Pool adds;
    l=0 Pool copy -> zpre f32 sbuf [128, (a, b, l) 512].
  - sqrt(x + 1e-4) on ACT -> zsb bf16 (x >= 0 so eps-add ~= max(x, eps)).
  - MM3 per (h, rl): lhsT = zsb slice [64 c, 64 (a,j)], rhs = W [64, 128 i],
    accumulate 8 rl in psum -> ps3 [(h,bank,blk,j)=v 128, 128 i].
  - relu (+ bias if nonzero) on DVE -> osb; one [128, 512B-row] store per sg.
"""

import sys

sys.path.insert(0, "/opt/trn_rl_repo")

import numpy as np

import concourse.bacc as bacc
import concourse.mybir as mybir
import concourse.tile as tile
from concourse import bass2jax

B, V, P, CIN, R, COUT = 8, 4096, 32, 64, 2, 128
NSH, NDEG = 16, 4
VSG = 128            # v's per supergroup
NSG = V // VSG       # 32 supergroups
NCHUNK = VSG // 4    # 32 chunks of 4 v's
SGI = VSG * P        # 4096 gather indices per supergroup
GSZ = 4096           # indices per dma_gather op
KGRP = 4             # supergroups per kbd/patch tile refill
PREGATHER = True     # host-side signal[pidx]: dense stream vs random gather
BF16 = mybir.dt.bfloat16
F32 = mybir.dt.float32
I16 = mybir.dt.int16

_CACHE = {}


def _dma_gather_any(eng, out_ap, in_ap, idxs_ap, num_idxs, elem_size,
                    single_packet=True):
    """bass.dma_gather minus the elem_size%256 assert (the Q7 ucode only
    requires the source ROW STRIDE to be a 256B multiple; the bytes read per
    row are free). in_ap's outer stride (elem_step) must be 256B-aligned."""
    from concourse import ap_utils
    from concourse.bass import MemorySpace

    assert idxs_ap.dtype == I16
    assert in_ap.space == MemorySpace.DRAM
    assert in_ap.dtype == out_ap.dtype
    elem_step = in_ap.ap[0][0]
    stride_bytes = elem_step * mybir.dt.size(in_ap.dtype)
    assert stride_bytes % 256 == 0 and stride_bytes // 256 < 256
    assert ap_utils.ap_is_contiguous(out_ap.ap[1:])
    assert ap_utils.ap_is_contiguous(idxs_ap.ap[1:])
    assert in_ap.ap[-1][1] == out_ap.ap[-1][1] == elem_size
    assert out_ap.ap[0][1] * out_ap.ap[1][1] == ((num_idxs + 127) // 128) * 128

    _in_ap = eng.lower_ap_dma(in_ap, for_custom_bir_dma=True)
    return eng.add_instruction(
        mybir.InstDMAGatherAnt(
            name=eng.bass.get_next_instruction_name(),
            ins=[*_in_ap, eng.lower_ap(idxs_ap),
                 eng.lower_val_access(eng.to_reg(num_idxs))],
            outs=[eng.lower_ap(out_ap)],
            transpose=False,
            num_idxs=num_idxs,
            elem_size=elem_size,
            stride_bytes_256=stride_bytes // 256,
            gen_mode=0,
            single_packet=single_packet,
            queue_num=0,
            sbuf_tokens_per_rank=0,
            sbuf_free_dim_per_rank=0,
            sbuf_free_dim_pad_per_rank=0,
            sbuf_byte_offset=0,
        ))


def _build_nc(nsg, with_bias):
    nc = bacc.Bacc("TRN2", target_bir_lowering=False, debug=False,
                   enable_asserts=False)
    vtot = nsg * VSG
    ngrp = nsg // KGRP
    kcols = KGRP * NCHUNK * R * NSH          # cols per j block: (sq, g, rn)
    if PREGATHER:
        pp = nc.dram_tensor("pp", [ngrp, 128, KGRP * NCHUNK * CIN], BF16,
                            kind="ExternalInput")
    else:
        sig = nc.dram_tensor("sig", [V, 128], BF16, kind="ExternalInput")
        idx = nc.dram_tensor("idx", [128, (SGI // 16) * nsg], I16,
                             kind="ExternalInput")
    kre = nc.dram_tensor("kre", [ngrp, 128, kcols], BF16,
                         kind="ExternalInput")
    wsb = nc.dram_tensor("wsb", [128, 8 * COUT], BF16, kind="ExternalInput")
    bia = nc.dram_tensor("bia", [1, COUT], F32, kind="ExternalInput")
    outd = nc.dram_tensor("outd", [vtot, COUT], F32, kind="ExternalOutput")

    AF = mybir.ActivationFunctionType
    ALU = mybir.AluOpType
    with tile.TileContext(nc) as tc:
        with (
            tc.tile_pool(name="const", bufs=1) as constp,
            tc.tile_pool(name="kbd", bufs=4) as stgp,
            tc.tile_pool(name="patches", bufs=4) as patp,
            tc.tile_pool(name="ysq", bufs=3) as ysqp,
            tc.tile_pool(name="zpre", bufs=3) as zprep,
            tc.tile_pool(name="zsb", bufs=4) as zsbp,
            tc.tile_pool(name="ytmp", bufs=3) as ytmpp,
            tc.tile_pool(name="osb", bufs=4) as osbp,
            tc.tile_pool(name="ps1", bufs=3, space="PSUM") as ps1p,
            tc.tile_pool(name="ps3", bufs=2, space="PSUM") as ps3p,
        ):
            w_t = constp.tile([128, 8 * COUT], BF16, tag="w")
            nc.sync.dma_start(w_t[:], wsb.ap())
            if not PREGATHER:
                idx_t = constp.tile([128, (SGI // 16) * nsg], I16, tag="idx")
                nc.sync.dma_start(idx_t[:], idx.ap())
            if with_bias:
                bias_t = constp.tile([1, COUT], F32, tag="bias")
                nc.sync.dma_start(bias_t[:], bia.ap())

            eps_t = constp.tile([128, 1], F32, tag="eps")
            nc.vector.memset(eps_t[:], 1e-4)

            stg_tiles = {}

            zsb2s = [zsbp.tile([128, 8 * COUT], BF16, tag="zsb2",
                               name=f"zsb2_{i}") for i in range(3)]
            for i in range(3):
                nc.vector.memset(zsb2s[i][:], 0.0)

            pat_tiles = {}

            def issue_pat(g, split=False):
                t = patp.tile([128, KGRP, NCHUNK, CIN], BF16, tag="pat",
                              name=f"pat{g % 4}")
                pat_tiles[g] = t
                if split:
                    for s in range(KGRP):
                        nc.sync.dma_start(t[:, s], pp.ap()[g].rearrange(
                            "p (sq r) -> p sq r", sq=KGRP)[:, s])
                else:
                    nc.sync.dma_start(t[:], pp.ap()[g])

            def issue_kbd(g):
                t = stgp.tile([128, kcols], BF16, tag="stg",
                              name=f"stg{g % 4}")
                stg_tiles[g] = t
                nc.sync.dma_start(t[:], kre.ap()[g])

            if PREGATHER:
                issue_pat(0, split=True)
                issue_kbd(0)
                issue_pat(1)
                issue_kbd(1)
                issue_pat(2)
                issue_kbd(2)
            else:
                issue_kbd(0)
                issue_kbd(1)
                issue_kbd(2)

            prev_store = None
            for sg in range(nsg):
                grp, sq = sg // KGRP, sg % KGRP
                if sq == 0 and grp + 3 < ngrp:
                    issue_kbd(grp + 3)
                stg_r = stg_tiles[grp][:, :].rearrange(
                    "p (sq g rn) -> p sq g rn", sq=KGRP, g=NCHUNK)

                # --- patches ----------------------------------------------
                if PREGATHER:
                    if sq == 0 and grp + 3 < ngrp:
                        issue_pat(grp + 3)
                    pat = pat_tiles[grp][:, sq]
                else:
                    patg = patp.tile([128, NCHUNK, CIN], BF16, tag="pat")
                    pat_r = patg[:, :, :].rearrange("p (t u) c -> p t u c",
                                                    t=1)
                    _dma_gather_any(
                        nc.gpsimd, pat_r[:, 0], sig.ap()[:, 0:CIN],
                        idx_t[:, (SGI // 16) * sg:(SGI // 16) * (sg + 1)],
                        GSZ, CIN, single_packet=False)
                    pat = patg[:, :, :]

                # store for the previous sg (data long ready; avoids
                # blocking any queue on this sg's compute chain)
                if prev_store is not None:
                    nc.gpsimd.dma_start(*prev_store)
                    prev_store = None

                # --- MM1: chunk g = 16h + 8b + blk -> ps1[b][64h, 128blk] -
                ps1 = [ps1p.tile([128, 1024], F32, tag="ps1",
                                 name=f"ps1_{sg}_{b}") for b in range(2)]
                for b in range(2):
                    for h in range(2):
                        for blk in range(8):
                            g = 16 * h + 8 * b + blk
                            for j in range(4):
                                nc.tensor.matmul(
                                    ps1[b][64 * h:64 * (h + 1),
                                           128 * blk + 32 * j:
                                           128 * blk + 32 * (j + 1)],
                                    pat[32 * j:32 * (j + 1), g, :],
                                    stg_r[32 * j:32 * (j + 1), sq, g, :],
                                    start=True, stop=True,
                                    tile_position=(32 * j, 64 * h))

                # --- square: ACT takes 1.5 banks; DVE squares the last
                # half-bank via a bf16 copy (one PSUM input) + 2x-mode mult -
                ysq = ysqp.tile([128, 2048], BF16, tag="ysq")
                nc.scalar.activation(ysq[:, 0:1024], ps1[0][:], AF.Square)
                nc.scalar.activation(ysq[:, 1024:1536],
                                     ps1[1][:, 0:512], AF.Square)
                ytmp = ytmpp.tile([128, 512], BF16, tag="ytmp")
                nc.vector.tensor_copy(ytmp[:], ps1[1][:, 512:1024])
                nc.vector.tensor_mul(ysq[:, 1536:2048], ytmp[:], ytmp[:])

                # --- degree sums over n-windows ---------------------------
                # ysq free: (a=(b,blk) 16, bb=(j,r) 8, n 16)
                zpre = zprep.tile([128, 512], F32, tag="zpre")
                ysq_r = ysq[:, :].rearrange("p (a bb n) -> p a bb n",
                                            a=16, bb=8)
                zpre_r = zpre[:, :].rearrange("p (a bb l) -> p a bb l",
                                              a=16, bb=8)
                # l=0: plain copy (Pool)
                nc.gpsimd.tensor_copy(zpre_r[:, :, :, 0], ysq_r[:, :, :, 0])
                # l=1: two adds (Pool)
                nc.gpsimd.tensor_tensor(zpre_r[:, :, :, 1],
                                        ysq_r[:, :, :, 1],
                                        ysq_r[:, :, :, 2], ALU.add)
                nc.gpsimd.tensor_tensor(zpre_r[:, :, :, 1],
                                        zpre_r[:, :, :, 1],
                                        ysq_r[:, :, :, 3], ALU.add)
                # l=2: DVE window reduce
                nc.vector.reduce_sum(
                    zpre_r[:, :, :, 2], ysq_r[:, :, :, 4:9],
                    axis=mybir.AxisListType.X)
                # l=3: six adds (Pool)
                nc.gpsimd.tensor_tensor(zpre_r[:, :, :, 3],
                                        ysq_r[:, :, :, 9],
                                        ysq_r[:, :, :, 10], ALU.add)
                for n in range(11, 16):
                    nc.gpsimd.tensor_tensor(zpre_r[:, :, :, 3],
                                            zpre_r[:, :, :, 3],
                                            ysq_r[:, :, :, n], ALU.add)

                # --- sqrt once on ACT, then copy each half into the
                # block-diag zsb2[(h,c), (r,l,v)] (off-block zeros persist
                # in the pinned tiles, so each rl slice is a [128, 128]
                # single-free-dim lhsT with K=(h,c)) ------------------------
                zsb = zsbp.tile([128, 512], BF16, tag="zsb")
                nc.scalar.activation(zsb[:], zpre[:], AF.Sqrt)
                zsb2 = zsb2s[sg % 3]
                zsb_h = zsb[:, :].rearrange("p (a j r l) -> p a j r l",
                                            a=16, j=4, r=2)
                zsb2_h = zsb2[:, :].rearrange(
                    "p (r l v2 a j) -> p a j r l v2", r=2, l=4, v2=2, a=16)
                nc.vector.tensor_copy(zsb2_h[0:64, :, :, :, :, 0],
                                      zsb_h[0:64])
                nc.gpsimd.tensor_copy(zsb2_h[64:128, :, :, :, :, 1],
                                      zsb_h[64:128])

                # --- MM3: contract (h, c) x 8 rl against duplicated W -----
                ps3 = ps3p.tile([128, COUT], F32, tag="ps3")
                for rl in range(8):
                    nc.tensor.matmul(
                        ps3[:, :],
                        zsb2[:, COUT * rl:COUT * (rl + 1)],
                        w_t[:, COUT * rl:COUT * (rl + 1)],
                        start=(rl == 0), stop=(rl == 7),
                        skip_group_check=True)

                # --- bias + relu; store deferred one sg -------------------
                osb = osbp.tile([128, COUT], F32, tag="osb")
                if with_bias:
                    nc.vector.tensor_add(
                        osb[:], ps3[:],
                        bias_t[:, :].broadcast(0, 128))
                    nc.vector.tensor_scalar_max(osb[:], osb[:], 0.0)
                else:
                    nc.gpsimd.tensor_scalar_max(osb[:], ps3[:], 0.0)
                prev_store = (outd.ap()[VSG * sg:VSG * (sg + 1), :], osb[:])
            nc.gpsimd.dma_start(*prev_store)

    nc.compile()
    return nc


def _prep_inputs_core(b, signal, patches_idx, conv_kernel, kernel_weights,
                      biases, nsg):
    bf = mybir.dt.np(BF16)
    ngrp = nsg // KGRP
    # kre[grp, j, p, (sq, g, rn)] = conv_kernel[b, (grp*KGRP+sq)*128+4g+j, p, rn]
    k = conv_kernel[b].reshape(ngrp, KGRP, NCHUNK, 4, P, R * NSH)
    kre = np.ascontiguousarray(
        k.transpose(0, 3, 4, 1, 2, 5)).reshape(
        ngrp, 128, KGRP * NCHUNK * R * NSH).astype(bf)
    # wsb[c + 64*dup, rl*128 + i] = kernel_weights[i, c, r, l], rl = 4r + l
    w = kernel_weights.transpose(2, 3, 1, 0).reshape(8, CIN, COUT)
    wrow = np.ascontiguousarray(w.transpose(1, 0, 2)).reshape(CIN, 8 * COUT)
    wsb = np.concatenate([wrow, wrow], axis=0).astype(bf)
    bia = biases.reshape(1, COUT).astype(np.float32)
    out = {"kre": kre, "wsb": wsb, "bia": bia}
    if PREGATHER:
        # pp[grp, 32j+p, (sq, g, c)] = signal[b][pidx[v=(grp*KGRP+sq)*128+4g+j, p]]
        pb = signal[b].astype(bf)[patches_idx[b, :, :, 1]]   # [V, P, C] bf16
        pb = pb.reshape(ngrp, KGRP, NCHUNK, 4, P, CIN)
        out["pp"] = np.ascontiguousarray(
            pb.transpose(0, 3, 4, 1, 2, 5)).reshape(
            ngrp, 128, KGRP * NCHUNK * CIN)
    else:
        sig = np.zeros((V, 128), dtype=bf)
        sig[:, :CIN] = signal[b].astype(bf)
        out["sig"] = sig
        out["idx"] = _fix_idx_wrap(
            patches_idx[b, :, :, 1].astype(np.int16).reshape(-1))
    return out


def _fix_idx_wrap(pidx_flat):
    # wrap order is per gather op: each op's GSZ idxs wrapped into 16
    # partitions independently.
    blk = pidx_flat.reshape(-1, GSZ // 16, 16)   # [ops, GSZ/16, 16]
    out = np.ascontiguousarray(
        blk.transpose(0, 2, 1).transpose(1, 0, 2)).reshape(16, -1)
    return np.tile(out, (8, 1))


def _make_runner(nc, n_cores=8):
    import jax
    from jax.sharding import Mesh, PartitionSpec
    from jax.experimental.shard_map import shard_map

    bass2jax.install_neuronx_cc_hook()
    partition_name = (nc.partition_id_tensor.name
                      if nc.partition_id_tensor else None)
    in_names, out_names, out_avals, zero_outs = [], [], [], []
    for alloc in nc.m.functions[0].allocations:
        if not isinstance(alloc, mybir.MemoryLocationSet):
            continue
        name = alloc.memorylocations[0].name
        if alloc.kind == "ExternalInput":
            if name != partition_name:
                in_names.append(name)
        elif alloc.kind == "ExternalOutput":
            out_names.append(name)
            shape = tuple(alloc.tensor_shape)
            dtype = mybir.dt.np(alloc.dtype)
            out_avals.append(jax.core.ShapedArray(shape, dtype))
            zero_outs.append(np.zeros(shape, dtype))
    n_params, n_outs = len(in_names), len(out_avals)
    in_names_all = list(in_names) + list(out_names)
    if partition_name is not None:
        in_names_all.append(partition_name)

    def _body(*args):
        operands = list(args)
        if partition_name is not None:
            operands.append(bass2jax.partition_id_tensor())
        outs = bass2jax._bass_exec_p.bind(
            *operands, out_avals=tuple(out_avals),
            in_names=tuple(in_names_all), out_names=tuple(out_names),
            lowering_input_output_aliases=(),
            sim_require_finite=True, sim_require_nnan=True, nc=nc)
        return tuple(outs)

    donate = tuple(range(n_params, n_params + n_outs))
    devices = jax.devices()[:n_cores]
    mesh = Mesh(np.asarray(devices), ("core",))
    sharded = jax.jit(
        shard_map(_body, mesh=mesh,
                  in_specs=(PartitionSpec("core"),) * (n_params + n_outs),
                  out_specs=(PartitionSpec("core"),) * n_outs,
                  check_rep=False),
        donate_argnums=donate, keep_unused=True)

    def run_fn(in_maps):
        import jax
        per_core = [[np.asarray(m[nm]) for nm in in_names] for m in in_maps]
        concat_in = [
            np.concatenate([per_core[c][i] for c in range(n_cores)], axis=0)
            for i in range(n_params)]
        concat_zeros = [
            np.zeros((n_cores * z.shape[0], *z.shape[1:]), z.dtype)
            for z in zero_outs]
        out_arrs = sharded(*concat_in, *concat_zeros)
        jax.block_until_ready(out_arrs)
        return [
            {nm: np.asarray(out_arrs[i]).reshape(n_cores, *out_avals[i].shape)[c]
             for i, nm in enumerate(out_names)}
            for c in range(n_cores)]

    return run_fn


def kernel(signal, patches_idx, conv_kernel, kernel_weights, biases):
    with_bias = bool(np.any(biases))
    key = ("k", NSG, with_bias)
    if key not in _CACHE:
        nc = _build_nc(NSG, with_bias)
        _CACHE[key] = (nc, _make_runner(nc))
    nc, run = _CACHE[key]

    in_maps = []
    for b in range(B):
        m = _prep_inputs_core(b, signal, patches_idx, conv_kernel,
                              kernel_weights, biases, NSG)
        in_maps.append(m)

    results = run(in_maps)
    out = np.stack([results[b]["outd"] for b in range(B)], axis=0)
    return out.astype(np.float32)
